# revision 2
# baseline (speedup 1.0000x reference)
"""Causal multi-head attention for Trainium2, head-sharded across 8 NeuronCores.

v2: transposed-PV design.  Scores are computed as ST[C, c] (C on partitions)
exactly like v1, but the PV contraction streams V as the moving operand so
z lands TRANSPOSED: zT[c, h] with the sequence position c on partitions.
That makes the softmax denominator a per-partition scalar, so normalization
is a single DVE tensor_scalar (no PE broadcast matmuls, no bc copies), and
PV matmuls stream only 64 columns per [128 c x 128 C] block instead of 512.
A cheap PE transpose per 128-row output tile restores z to [h, c] for the
output projection.

Per-core PE column budget (cycles @ 2.4 GHz):
    proj Q/K/V   49152      scores  34816      PV      17408
    denominators   ~272     transposes 2048    outproj 16384
    total ~120k cycles ~= 50 us (vs 139k = 58 us for v1).

PSUM (8 banks): scores 3 x [128,512]f32, zT 1 x [128,16,64]f32 (2 banks,
chunk-parity halves), transpose+denominator 1 bank ([128,1024]bf16; cols
768:800 bitcast to [128,16]f32 hold the denominators), proj 1, outproj 1.
"""

import numpy as np
import ml_dtypes

import concourse.bacc as bacc
import concourse.mybir as mybir
import concourse.tile as tile
from concourse import bass_utils

BATCH, SEQ, E, NH, H = 1, 2048, 1024, 16, 64
NCORES = 8
HPC = NH // NCORES          # heads per core
H2 = HPC * H                # 128
CS = 512                    # chunk width (c columns per score matmul)
NCS = SEQ // CS             # 4
NKT = E // 128              # 8 k-tiles over embed
NCT = SEQ // 128            # 16 C-tiles over sequence
SCALE = 1.0 / np.sqrt(H)
F32 = mybir.dt.float32
BF16 = mybir.dt.bfloat16
NPBF16 = ml_dtypes.bfloat16

_built = None

# ---- schedule knobs -------------------------------------------------------
DEPTH = 3            # PV lags scores by this many (cs, ct) blocks
ESB_BUFS = 14        # exp-output tiles in flight
ZN_BUFS = 4          # normalized zT tiles
Z2_BUFS = 4          # transposed z tiles
OSB_BUFS = 3         # outproj sbuf tiles
POPN = (2, 1)        # pop POPN[0] filler units every POPN[1] blocks
TR_SLOTS = 3         # transpose slots in the trden bank
OC_ENGINES = ("dve", "act")  # outproj copy engine rotation
QK_ENGINE = "dve"    # qk proj copy engine
V_ENGINE = "dve"     # v proj copy engine
Z2C_ENGINE = "dve"   # transpose psum -> sbuf copy engine
V_IN_TRDEN = False   # V-proj psum in the trden bank vs shared ps_u
INTER_LEAD = 2       # chunk-2 blocks before chunk-3 interleave starts
INTER_N = 6          # chunk-3 blocks interleaved into chunk 2
FMAX = 6             # adaptive pop: keep len(filler) near this
WARMUP = 26          # garbage matmuls before the first DMA lands (pstate ramp)
TR_DELAY = 2         # blocks between a sub's diag PV and its transpose pop
OUT_DELAY = 3        # blocks between transpose and outproj pops


def _build(stage=5):
    nc = bacc.Bacc("TRN2", target_bir_lowering=False, debug=False)

    xT_d = nc.dram_tensor("xT", [E, SEQ], BF16, kind="ExternalInput").ap()
    wq_d = nc.dram_tensor("wq", [128, NKT, H2], BF16, kind="ExternalInput").ap()
    wk_d = nc.dram_tensor("wk", [128, NKT, H2], BF16, kind="ExternalInput").ap()
    wv_d = nc.dram_tensor("wv", [128, NKT, H2], BF16, kind="ExternalInput").ap()
    wo_d = nc.dram_tensor("wo", [H2, E], BF16, kind="ExternalInput").ap()
    masks_d = nc.dram_tensor("masksb", [128, 128], BF16, kind="ExternalInput").ap()
    ident_d = nc.dram_tensor("ident", [128, 128], F32, kind="ExternalInput").ap()
    onesc_d = nc.dram_tensor("onesc", [128, 4], BF16, kind="ExternalInput").ap()
    vcol_d = nc.dram_tensor("vcol", [128, HPC, NCT, 2], BF16,
                            kind="ExternalInput").ap()
    out_d = nc.dram_tensor("out", [SEQ, E], BF16, kind="ExternalOutput").ap()

    def _body(tc):
        with (
            tc.tile_pool(name="persist", bufs=1) as persist,
            tc.tile_pool(name="work", bufs=4) as work,
            tc.tile_pool(name="ps_s", bufs=2, space="PSUM") as ps_s,
            tc.tile_pool(name="ps_zt", bufs=1, space="PSUM") as ps_zt,
            tc.tile_pool(name="ps_p", bufs=1, space="PSUM") as ps_p,
            tc.tile_pool(name="ps_o", bufs=1, space="PSUM") as ps_o,
        ):
            # ---- resident tensors -------------------------------------
            xT = persist.tile([128, NKT, SEQ], BF16)
            wq = persist.tile([128, NKT, H2], BF16)
            wk = persist.tile([128, NKT, H2], BF16)
            wv = persist.tile([128, NKT, H2], BF16)
            wo = persist.tile([128, E], BF16)
            mask_sb = persist.tile([128, 128], BF16)
            ident = persist.tile([128, 128], F32)
            onesc = persist.tile([128, 4], BF16)
            QT2 = persist.tile([128, SEQ], BF16)
            KT2 = persist.tile([128, SEQ], BF16)
            V1 = persist.tile([128, HPC, NCT, H + 2], BF16)

            # persistent PSUM accumulators (allocated once, managed by
            # subtile deps): zT [c, slot, h] and the transpose/denominator
            # bank.  slot = parity*8 + sub*2 + hh.
            # zT accumulator, 2 banks: slot idx = sub*2 + hh, bank idx//4,
            # 65 columns per slot (64 z + the ones-column denominator).
            # start=True zeroes a WHOLE psum bank, so within a PV group only
            # the first matmul touching each bank may use start=True.
            zt = ps_zt.tile([128, 2, CS], F32, name="zt")

            def ztsl(s, hh):
                idx = s * 2 + hh
                return zt[:, idx // 4, (idx % 4) * 65:(idx % 4) * 65 + 65]

            # warmup: garbage matmuls on a never-initialized tile keep the
            # PE busy during the initial DMA wait so the p-state ramp (the
            # cost model halves PE speed for the first 3us of a busy run)
            # is paid on discarded work
            if WARMUP:
                wu = persist.tile([128, 128], BF16)
                nc.gpsimd.memset(wu[:], 0.0)
                wu_ps = ps_p.tile([128, CS], F32, tag="p", name="wu_ps")
                for w in range(WARMUP):
                    nc.tensor.matmul(wu_ps[:, 0:128], wu[:], wu[:],
                                     start=True, stop=True,
                                     skip_group_check=True)

            # ---- input DMAs (issue order tracks first use) ------------
            xTv = xT_d.rearrange("(k p) c -> p k c", p=128)
            nc.sync.dma_start(wq[:, 0:2, :], wq_d[:, 0:2, :])
            nc.sync.dma_start(xT[:, 0:2, 0:CS], xTv[:, 0:2, 0:CS])
            nc.sync.dma_start(wq[:, 2:4, :], wq_d[:, 2:4, :])
            nc.sync.dma_start(xT[:, 2:4, 0:CS], xTv[:, 2:4, 0:CS])
            nc.sync.dma_start(wk[:, 0:4, :], wk_d[:, 0:4, :])
            nc.sync.dma_start(wq[:, NKT // 2:, :], wq_d[:, NKT // 2:, :])
            nc.sync.dma_start(xT[:, NKT // 2:, 0:CS], xTv[:, NKT // 2:, 0:CS])
            nc.sync.dma_start(wk[:, 4:, :], wk_d[:, 4:, :])
            nc.sync.dma_start(wv[:], wv_d[:])
            nc.sync.dma_start(mask_sb[:], masks_d[:])
            nc.sync.dma_start(ident[:], ident_d[:])
            nc.sync.dma_start(onesc[:], onesc_d[:])
            nc.sync.dma_start(V1[:, :, :, H:H + 2], vcol_d[:])
            nc.sync.dma_start(xT[:, :, CS:2 * CS], xTv[:, :, CS:2 * CS])
            nc.sync.dma_start(wo[:], wo_d[:])
            for cc in range(2, NCS):
                nc.sync.dma_start(xT[:, :, cc * CS:(cc + 1) * CS],
                                  xTv[:, :, cc * CS:(cc + 1) * CS])

            def copy_eng(which):
                return {"act": nc.scalar, "dve": nc.vector,
                        "pool": nc.gpsimd}[which]

            def do_copy(which, dst, src):
                if which == "act":
                    nc.scalar.activation(dst, src,
                                         mybir.ActivationFunctionType.Copy)
                else:
                    copy_eng(which).tensor_copy(dst, src)

            # ---- projections ------------------------------------------
            qk_live = {}

            def emit_qk_proj(cc, w_sb, dstT, half, ap_slot=None, defer=False):
                c0, c1 = cc * CS, (cc + 1) * CS
                if half == 0:
                    if ap_slot is None:
                        p_ps = ps_p.tile([128, CS], F32, tag="p", name="p_ps")
                    else:
                        p_ps = ap_slot
                    qk_live[(cc, id(dstT))] = p_ps
                else:
                    p_ps = qk_live.pop((cc, id(dstT)))
                for k in range(4 * half, 4 * half + 4):
                    nc.tensor.matmul(
                        p_ps[:], w_sb[:, k, :], xT[:, k, c0:c1],
                        start=(k == 0), stop=(k == NKT - 1),
                    )
                if half == 1:
                    # the copy pops as its own (delayed) unit so it never
                    # waits at its engine's queue head for the matmuls
                    cp = lambda: do_copy(QK_ENGINE, dstT[:, c0:c1], p_ps[:])
                    if defer:
                        filler.insert(0, (("proj", cc), cur_i[0] + 1, cp))
                    else:
                        cp()

            def emit_v_tile(cc, i, ap_slot=None, defer=False):
                ct = 4 * cc + i
                if ap_slot is not None:
                    v_ps = ap_slot
                elif V_IN_TRDEN:
                    v_ps = vregs[i % 2]
                else:
                    v_ps = ps_p.tile([128, CS], F32, tag="p", name="v_ps")
                for k in range(NKT):
                    nc.tensor.matmul(
                        v_ps[:, 0:H2], xT[:, k, ct * 128:(ct + 1) * 128],
                        wv[:, k, :],
                        start=(k == 0), stop=(k == NKT - 1),
                        skip_group_check=True,
                    )
                cp = lambda: do_copy(V_ENGINE, V1[:, :, ct, 0:H],
                                     v_ps[:, 0:H2])
                if defer:
                    filler.insert(0, (("proj", cc), cur_i[0] + 1, cp))
                else:
                    cp()

            def proj_units(cc, head=False):
                # chunk-0 projections borrow the idle score-psum halves so
                # the initial burst double-buffers; later chunks trickle
                # through the shared 1-bank pool
                units = []
                if head:
                    sa = ps_s.tile([128, 2, CS], F32, tag="s", name="s2")
                    sb = ps_s.tile([128, 2, CS], F32, tag="s", name="s2")
                    vb = ps_o.tile([128, CS], F32, tag="o", name="o_ps")
                    units.append(lambda: emit_qk_proj(cc, wq, QT2, 0,
                                                      sa[:, 0, :]))
                    units.append(lambda: emit_qk_proj(cc, wq, QT2, 1))
                    units.append(lambda: emit_qk_proj(cc, wk, KT2, 0,
                                                      sb[:, 0, :]))
                    units.append(lambda: emit_qk_proj(cc, wk, KT2, 1))
                    units.append(lambda: emit_v_tile(cc, 0, sa[:, 1, :]))
                    units.append(lambda: emit_v_tile(cc, 1, sb[:, 1, :]))
                    units.append(lambda: emit_v_tile(cc, 2, vb))
                    units.append(lambda: emit_v_tile(cc, 3))
                else:
                    units.append(lambda: emit_qk_proj(cc, wq, QT2, 0))
                    units.append(lambda: emit_qk_proj(cc, wq, QT2, 1,
                                                      defer=True))
                    units.append(lambda: emit_qk_proj(cc, wk, KT2, 0))
                    units.append(lambda: emit_qk_proj(cc, wk, KT2, 1,
                                                      defer=True))
                    for i in range(4):
                        units.append(lambda i=i: emit_v_tile(cc, i,
                                                             defer=True))
                return [(("proj", cc), 0, u) for u in units]

            # ---- global block pipeline --------------------------------
            # Score emission order: chunk 3's early C-tiles are interleaved
            # into chunk 2 (chunk 3 is exp-heavy; its scores can run early,
            # buffered in SBUF).  PV order stays natural per chunk so the
            # single-set zT accumulator never sees two chunks at once.
            pv_list = []
            for cs in range(NCS):
                for ct in range(4 * cs + 4):
                    pv_list.append((cs, ct))
            NB = len(pv_list)
            score_list = list(pv_list)
            smap = {b: k for k, b in enumerate(score_list)}

            exp_tiles = {}
            filler = []     # entries: (tag, min_block, fn)
            oc_idx = [0]
            tr_count = [0]
            drain = [False]
            cur_i = [0]

            def do_scores(blk):
                cs, ct = blk
                d = ct - 4 * cs
                off = 128 * d if d > 0 else 0
                n = CS - off
                s2 = ps_s.tile([128, 2, CS], F32, tag="s", name="s2")
                for hh in range(HPC):
                    h0 = hh * H
                    nc.tensor.matmul(
                        s2[:, hh, 0:n],
                        QT2[h0:h0 + H, ct * 128:(ct + 1) * 128],
                        KT2[h0:h0 + H, cs * CS + off:(cs + 1) * CS],
                        start=True, stop=True,
                    )
                e2 = work.tile([128, 2, CS], BF16, tag="exp",
                               bufs=ESB_BUFS, name="e2")
                nc.scalar.activation(
                    e2[:, :, 0:n], s2[:, :, 0:n],
                    mybir.ActivationFunctionType.Exp, scale=SCALE,
                )
                if d >= 0:
                    for hh in range(HPC):
                        nc.gpsimd.tensor_tensor(
                            e2[:, hh, 0:128], e2[:, hh, 0:128], mask_sb[:],
                            op=mybir.AluOpType.mult,
                        )
                exp_tiles[blk] = (e2, off, n)

            def emit_norm(cs, s):
                # recip for this sub-pair group is emitted by do_pv (it only
                # depends on the denominator columns); here: normalize both
                # heads into one [c, h2] sbuf tile, then queue transpose and
                # outproj units
                rsb = norm_recips[cs]
                zn = work.tile([128, H2], F32, tag="zn", bufs=ZN_BUFS,
                               name="zn")
                # one head normalizes on DVE, the other on ACT (activation
                # Copy with a per-partition scale AP) so the two run in
                # parallel -- this chain gates the kernel tail
                nc.vector.tensor_scalar(
                    zn[:, 0:H], ztsl(s, 0)[:, 0:H],
                    rsb[:, s * 2:s * 2 + 1], None,
                    op0=mybir.AluOpType.mult,
                )
                nc.vector.tensor_scalar(
                    zn[:, H:H2], ztsl(s, 1)[:, 0:H],
                    rsb[:, s * 2 + 1:s * 2 + 2], None,
                    op0=mybir.AluOpType.mult,
                )
                filler.insert(0, (("tr", cs), cur_i[0] + TR_DELAY,
                                  make_tr(cs, s, zn)))

            def make_tr(cs, s, zn):
                def run():
                    tr_ps = ps_o.tile([128, CS], F32, tag="o", name="o_ps")
                    nc.tensor.transpose(tr_ps[:, 0:128], zn[:], ident[:])
                    z2 = work.tile([128, 128], BF16, tag="z2", bufs=Z2_BUFS,
                                   name="z2")
                    do_copy("dve" if drain[0] else Z2C_ENGINE, z2[:],
                            tr_ps[:, 0:128])
                    o_sb = work.tile([128, E], BF16, tag="o", bufs=OSB_BUFS,
                                     name="o_sb")
                    filler.append((("out", cs), cur_i[0] + OUT_DELAY,
                                   make_out(cs, s, z2, 0, o_sb)))
                    filler.append((("out", cs), cur_i[0] + OUT_DELAY + 1,
                                   make_out(cs, s, z2, 1, o_sb)))
                return run

            def make_out(cs, s, z2, et, o_sb):
                def run():
                    last = cs == NCS - 1 and s == 3
                    if drain[0]:
                        # scores are finished: borrow the idle score banks so
                        # the final outproj chain pipelines wider
                        sl = ps_s.tile([128, 2, CS], F32, tag="s", name="s2")
                        o_ps = sl[:, et, :]
                        engs = ("act", "dve")
                    else:
                        o_ps = ps_o.tile([128, CS], F32, tag="o", name="o_ps")
                        engs = OC_ENGINES
                    nc.tensor.matmul(
                        o_ps[:], z2[:], wo[:, et * CS:(et + 1) * CS],
                        start=True, stop=True,
                    )
                    eng = engs[oc_idx[0] % len(engs)]
                    oc_idx[0] += 1
                    do_copy(eng, o_sb[:, et * CS:(et + 1) * CS], o_ps[:])
                    if last:
                        nc.sync.dma_start(
                            out_d[cs * CS + s * 128:cs * CS + (s + 1) * 128,
                                  et * CS:(et + 1) * CS],
                            o_sb[:, et * CS:(et + 1) * CS])
                    elif et == 1:
                        nc.sync.dma_start(
                            out_d[cs * CS + s * 128:cs * CS + (s + 1) * 128, :],
                            o_sb[:])
                return run

            norm_recips = {}

            def do_pv(blk):
                cs, ct = blk
                d = ct - 4 * cs
                off = 128 * d if d > 0 else 0
                diag = ct - 4 * cs  # sub index whose accumulation ends here
                # masked sub (s == d) last: its lhsT waits the DVE mask op
                subs = list(range(max(d, 0), 4))
                if d >= 0 and len(subs) > 1:
                    subs = subs[1:] + subs[:1]
                e2, off_, n = exp_tiles.pop(blk)
                bank_started = [False, False]
                for hh in range(HPC):
                    for s in subs:
                        lo = s * 128 - off
                        bank = (s * 2 + hh) // 4
                        st = False
                        if ct == 0 and not bank_started[bank]:
                            st = True
                            bank_started[bank] = True
                        nc.tensor.matmul(
                            ztsl(s, hh), e2[:, hh, lo:lo + 128],
                            V1[:, hh, ct, 0:H + 1],
                            start=st, stop=(ct == 4 * cs + s),
                            skip_group_check=True,
                        )
                if 0 <= diag < 4:
                    s = diag
                    # reciprocal granularity: subs {0,1} together, then 2, 3
                    groups = {1: (0, 1), 2: (2,), 3: (3,)}
                    if cs == 0:
                        groups = {0: (0,), 1: (1,), 2: (2,), 3: (3,)}
                    if s in groups:
                        g = groups[s]
                        lo_s, hi_s = g[0], g[-1]
                        rsb = norm_recips.get(cs)
                        if rsb is None:
                            rsb = work.tile([128, 8], F32, tag="recip",
                                            bufs=2, name="rsb")
                            norm_recips[cs] = rsb
                        for ss in g:
                            for hh in range(HPC):
                                nc.vector.reciprocal(
                                    rsb[:, ss * 2 + hh:ss * 2 + hh + 1],
                                    ztsl(ss, hh)[:, 64:65])
                        for ss in g:
                            emit_norm(cs, ss)

            def pop(nmax=1):
                npop = 0
                j = 0
                while j < len(filler) and npop < nmax:
                    tag, min_i, fn = filler[j]
                    if drain[0] or cur_i[0] >= min_i:
                        filler.pop(j)
                        fn()
                        npop += 1
                    else:
                        j += 1

            def force_proj(cs):
                j = 0
                while j < len(filler):
                    if filler[j][0] == ("proj", cs):
                        filler.pop(j)[2]()
                    else:
                        j += 1

            for i in range(NB):
                cur_i[0] = i
                cs, ct = pv_list[i]
                if ct == 0:
                    force_proj(cs)
                    if cs == 0:
                        for _, _, u in proj_units(0, head=True):
                            u()
                        filler.extend(proj_units(1))
                    elif cs == 1:
                        filler.extend(proj_units(2))
                        filler.extend(proj_units(3))
                do_scores(pv_list[i])
                npop = POPN[0]
                if len(filler) > FMAX:
                    npop += (len(filler) - FMAX + 1) // 2
                pop(npop)
                if i >= DEPTH:
                    do_pv(pv_list[i - DEPTH])
            drain[0] = True
            for i in range(NB - DEPTH, NB):
                do_pv(pv_list[i])
                pop(2)
            while filler:
                pop(1)

    with tile.TileContext(nc) as tc:
        _body(tc)
    nc.finalize()
    return nc


def _prep_inputs(x, W_Q, W_K, W_V, W_O):
    x = np.asarray(x, dtype=np.float32)
    W_Q = np.asarray(W_Q, dtype=np.float32)
    W_K = np.asarray(W_K, dtype=np.float32)
    W_V = np.asarray(W_V, dtype=np.float32)
    W_O = np.asarray(W_O, dtype=np.float32)

    xT = np.ascontiguousarray(x[0].T).astype(NPBF16)       # [E, SEQ]

    def swz(w):
        # [E, H2] -> [128(p), NKT(k), H2]
        return np.ascontiguousarray(
            w.reshape(NKT, 128, H2).transpose(1, 0, 2)).astype(NPBF16)

    in_maps = []
    for c in range(NCORES):
        a0, a1 = HPC * c, HPC * c + 1
        wq = swz(np.concatenate([W_Q[a0].T, W_Q[a1].T], axis=1))
        wk = swz(np.concatenate([W_K[a0].T, W_K[a1].T], axis=1))
        wv = swz(np.concatenate([W_V[a0].T, W_V[a1].T], axis=1))
        wo = np.ascontiguousarray(
            np.concatenate([W_O[a0].T, W_O[a1].T], axis=0)).astype(NPBF16)
        in_maps.append({"xT": xT, "wq": wq, "wk": wk, "wv": wv, "wo": wo,
                        "masksb": _MASKS, "ident": _IDENT, "onesc": _ONESC,
                        "vcol": _VCOL})
    return in_maps


_MASKS = (np.arange(128)[:, None] <= np.arange(128)[None, :]).astype(NPBF16)
_IDENT = np.eye(128, dtype=np.float32)
_ONESC = np.ones((128, 4), dtype=NPBF16)
_VCOL = np.ones((128, HPC, NCT, 2), dtype=NPBF16)


def _run(in_maps, trace=False):
    global _built
    if _built is None:
        _built = _build()
    res = bass_utils.run_bass_kernel_spmd(
        _built, in_maps, core_ids=list(range(NCORES)), trace=trace,
    )
    return res


def kernel(x, W_Q, W_K, W_V, W_O):
    in_maps = _prep_inputs(x, W_Q, W_K, W_V, W_O)
    res = _run(in_maps, trace=False)
    acc = np.zeros((SEQ, E), dtype=np.float64)
    for c in range(NCORES):
        acc += np.asarray(res.results[c]["out"], dtype=np.float64)
    return acc.astype(np.float32)[None, :, :]


def kernel_traced(x, W_Q, W_K, W_V, W_O):
    """Like kernel() but also returns a per-core exec-time estimate in ns."""
    in_maps = _prep_inputs(x, W_Q, W_K, W_V, W_O)
    exec_ns = None
    try:
        res = _run(in_maps, trace=True)
        exec_ns = res.exec_time_ns
    except Exception:
        res = _run(in_maps, trace=False)
    if exec_ns is None:
        from concourse.timeline_sim import TimelineSim
        exec_ns = int(TimelineSim(_built, trace=False).simulate())
    acc = np.zeros((SEQ, E), dtype=np.float64)
    for c in range(NCORES):
        acc += np.asarray(res.results[c]["out"], dtype=np.float64)
    return acc.astype(np.float32)[None, :, :], exec_ns


# revision 3
# speedup vs baseline: 1.0013x; 1.0013x over previous
"""Causal multi-head attention for Trainium2, head-sharded across 8 NeuronCores.

v2: transposed-PV design.  Scores are computed as ST[C, c] (C on partitions)
exactly like v1, but the PV contraction streams V as the moving operand so
z lands TRANSPOSED: zT[c, h] with the sequence position c on partitions.
That makes the softmax denominator a per-partition scalar, so normalization
is a single DVE tensor_scalar (no PE broadcast matmuls, no bc copies), and
PV matmuls stream only 64 columns per [128 c x 128 C] block instead of 512.
A cheap PE transpose per 128-row output tile restores z to [h, c] for the
output projection.

Per-core PE column budget (cycles @ 2.4 GHz):
    proj Q/K/V   49152      scores  34816      PV      17408
    denominators   ~272     transposes 2048    outproj 16384
    total ~120k cycles ~= 50 us (vs 139k = 58 us for v1).

PSUM (8 banks): scores 3 x [128,512]f32, zT 1 x [128,16,64]f32 (2 banks,
chunk-parity halves), transpose+denominator 1 bank ([128,1024]bf16; cols
768:800 bitcast to [128,16]f32 hold the denominators), proj 1, outproj 1.
"""

import numpy as np
import ml_dtypes

import concourse.bacc as bacc
import concourse.mybir as mybir
import concourse.tile as tile
from concourse import bass_utils

BATCH, SEQ, E, NH, H = 1, 2048, 1024, 16, 64
NCORES = 8
HPC = NH // NCORES          # heads per core
H2 = HPC * H                # 128
CS = 512                    # chunk width (c columns per score matmul)
NCS = SEQ // CS             # 4
NKT = E // 128              # 8 k-tiles over embed
NCT = SEQ // 128            # 16 C-tiles over sequence
SCALE = 1.0 / np.sqrt(H)
F32 = mybir.dt.float32
BF16 = mybir.dt.bfloat16
NPBF16 = ml_dtypes.bfloat16

_built = None

# ---- schedule knobs -------------------------------------------------------
DEPTH = 3            # PV lags scores by this many (cs, ct) blocks
ESB_BUFS = 14        # exp-output tiles in flight
ZN_BUFS = 4          # normalized zT tiles
Z2_BUFS = 4          # transposed z tiles
OSB_BUFS = 3         # outproj sbuf tiles
POPN = (2, 1)        # pop POPN[0] filler units every POPN[1] blocks
TR_SLOTS = 3         # transpose slots in the trden bank
OC_ENGINES = ("dve", "act")  # outproj copy engine rotation
QK_ENGINE = "dve"    # qk proj copy engine
V_ENGINE = "dve"     # v proj copy engine
Z2C_ENGINE = "dve"   # transpose psum -> sbuf copy engine
V_IN_TRDEN = False   # V-proj psum in the trden bank vs shared ps_u
INTER_LEAD = 2       # chunk-2 blocks before chunk-3 interleave starts
INTER_N = 6          # chunk-3 blocks interleaved into chunk 2
FMAX = 6             # adaptive pop: keep len(filler) near this
WARMUP = 26          # garbage matmuls before the first DMA lands (pstate ramp)
TR_DELAY = 1         # blocks between a sub's diag PV and its transpose pop
OUT_DELAY = 2        # blocks between transpose and outproj pops


def _build(stage=5):
    nc = bacc.Bacc("TRN2", target_bir_lowering=False, debug=False)

    xT_d = nc.dram_tensor("xT", [E, SEQ], BF16, kind="ExternalInput").ap()
    wq_d = nc.dram_tensor("wq", [128, NKT, H2], BF16, kind="ExternalInput").ap()
    wk_d = nc.dram_tensor("wk", [128, NKT, H2], BF16, kind="ExternalInput").ap()
    wv_d = nc.dram_tensor("wv", [128, NKT, H2], BF16, kind="ExternalInput").ap()
    wo_d = nc.dram_tensor("wo", [H2, E], BF16, kind="ExternalInput").ap()
    masks_d = nc.dram_tensor("masksb", [128, 128], BF16, kind="ExternalInput").ap()
    ident_d = nc.dram_tensor("ident", [128, 128], F32, kind="ExternalInput").ap()
    onesc_d = nc.dram_tensor("onesc", [128, 4], BF16, kind="ExternalInput").ap()
    vcol_d = nc.dram_tensor("vcol", [128, HPC, NCT, 2], BF16,
                            kind="ExternalInput").ap()
    out_d = nc.dram_tensor("out", [SEQ, E], BF16, kind="ExternalOutput").ap()

    def _body(tc):
        with (
            tc.tile_pool(name="persist", bufs=1) as persist,
            tc.tile_pool(name="work", bufs=4) as work,
            tc.tile_pool(name="ps_s", bufs=2, space="PSUM") as ps_s,
            tc.tile_pool(name="ps_zt", bufs=1, space="PSUM") as ps_zt,
            tc.tile_pool(name="ps_p", bufs=1, space="PSUM") as ps_p,
            tc.tile_pool(name="ps_o", bufs=1, space="PSUM") as ps_o,
        ):
            # ---- resident tensors -------------------------------------
            xT = persist.tile([128, NKT, SEQ], BF16)
            wq = persist.tile([128, NKT, H2], BF16)
            wk = persist.tile([128, NKT, H2], BF16)
            wv = persist.tile([128, NKT, H2], BF16)
            wo = persist.tile([128, E], BF16)
            mask_sb = persist.tile([128, 128], BF16)
            ident = persist.tile([128, 128], F32)
            onesc = persist.tile([128, 4], BF16)
            QT2 = persist.tile([128, SEQ], BF16)
            KT2 = persist.tile([128, SEQ], BF16)
            V1 = persist.tile([128, HPC, NCT, H + 2], BF16)

            # persistent PSUM accumulators (allocated once, managed by
            # subtile deps): zT [c, slot, h] and the transpose/denominator
            # bank.  slot = parity*8 + sub*2 + hh.
            # zT accumulator, 2 banks: slot idx = sub*2 + hh, bank idx//4,
            # 65 columns per slot (64 z + the ones-column denominator).
            # start=True zeroes a WHOLE psum bank, so within a PV group only
            # the first matmul touching each bank may use start=True.
            zt = ps_zt.tile([128, 2, CS], F32, name="zt")

            def ztsl(s, hh):
                idx = s * 2 + hh
                return zt[:, idx // 4, (idx % 4) * 65:(idx % 4) * 65 + 65]

            # warmup: garbage matmuls on a never-initialized tile keep the
            # PE busy during the initial DMA wait so the p-state ramp (the
            # cost model halves PE speed for the first 3us of a busy run)
            # is paid on discarded work
            if WARMUP:
                wu = persist.tile([128, 128], BF16)
                nc.gpsimd.memset(wu[:], 0.0)
                wu_ps = ps_p.tile([128, CS], F32, tag="p", name="wu_ps")
                for w in range(WARMUP):
                    nc.tensor.matmul(wu_ps[:, 0:128], wu[:], wu[:],
                                     start=True, stop=True,
                                     skip_group_check=True)

            # ---- input DMAs (issue order tracks first use) ------------
            xTv = xT_d.rearrange("(k p) c -> p k c", p=128)
            nc.sync.dma_start(wq[:, 0:2, :], wq_d[:, 0:2, :])
            nc.sync.dma_start(xT[:, 0:2, 0:CS], xTv[:, 0:2, 0:CS])
            nc.sync.dma_start(wq[:, 2:4, :], wq_d[:, 2:4, :])
            nc.sync.dma_start(xT[:, 2:4, 0:CS], xTv[:, 2:4, 0:CS])
            nc.sync.dma_start(wk[:, 0:4, :], wk_d[:, 0:4, :])
            nc.sync.dma_start(wq[:, NKT // 2:, :], wq_d[:, NKT // 2:, :])
            nc.sync.dma_start(xT[:, NKT // 2:, 0:CS], xTv[:, NKT // 2:, 0:CS])
            nc.sync.dma_start(wk[:, 4:, :], wk_d[:, 4:, :])
            nc.sync.dma_start(wv[:], wv_d[:])
            nc.sync.dma_start(mask_sb[:], masks_d[:])
            nc.sync.dma_start(ident[:], ident_d[:])
            nc.sync.dma_start(onesc[:], onesc_d[:])
            nc.sync.dma_start(V1[:, :, :, H:H + 2], vcol_d[:])
            nc.sync.dma_start(xT[:, :, CS:2 * CS], xTv[:, :, CS:2 * CS])
            nc.sync.dma_start(wo[:], wo_d[:])
            for cc in range(2, NCS):
                nc.sync.dma_start(xT[:, :, cc * CS:(cc + 1) * CS],
                                  xTv[:, :, cc * CS:(cc + 1) * CS])

            def copy_eng(which):
                return {"act": nc.scalar, "dve": nc.vector,
                        "pool": nc.gpsimd}[which]

            def do_copy(which, dst, src):
                if which == "act":
                    nc.scalar.activation(dst, src,
                                         mybir.ActivationFunctionType.Copy)
                else:
                    copy_eng(which).tensor_copy(dst, src)

            # ---- projections ------------------------------------------
            qk_live = {}

            def emit_qk_proj(cc, w_sb, dstT, half, ap_slot=None, defer=False):
                c0, c1 = cc * CS, (cc + 1) * CS
                if half == 0:
                    if ap_slot is None:
                        p_ps = ps_p.tile([128, CS], F32, tag="p", name="p_ps")
                    else:
                        p_ps = ap_slot
                    qk_live[(cc, id(dstT))] = p_ps
                else:
                    p_ps = qk_live.pop((cc, id(dstT)))
                for k in range(4 * half, 4 * half + 4):
                    nc.tensor.matmul(
                        p_ps[:], w_sb[:, k, :], xT[:, k, c0:c1],
                        start=(k == 0), stop=(k == NKT - 1),
                    )
                if half == 1:
                    # the copy pops as its own (delayed) unit so it never
                    # waits at its engine's queue head for the matmuls
                    cp = lambda: do_copy(QK_ENGINE, dstT[:, c0:c1], p_ps[:])
                    if defer:
                        filler.insert(0, (("proj", cc), cur_i[0] + 1, cp))
                    else:
                        cp()

            def emit_v_tile(cc, i, ap_slot=None, defer=False):
                ct = 4 * cc + i
                if ap_slot is not None:
                    v_ps = ap_slot
                elif V_IN_TRDEN:
                    v_ps = vregs[i % 2]
                else:
                    v_ps = ps_p.tile([128, CS], F32, tag="p", name="v_ps")
                for k in range(NKT):
                    nc.tensor.matmul(
                        v_ps[:, 0:H2], xT[:, k, ct * 128:(ct + 1) * 128],
                        wv[:, k, :],
                        start=(k == 0), stop=(k == NKT - 1),
                        skip_group_check=True,
                    )
                cp = lambda: do_copy(V_ENGINE, V1[:, :, ct, 0:H],
                                     v_ps[:, 0:H2])
                if defer:
                    filler.insert(0, (("proj", cc), cur_i[0] + 1, cp))
                else:
                    cp()

            def proj_units(cc, head=False):
                # chunk-0 projections borrow the idle score-psum halves so
                # the initial burst double-buffers; later chunks trickle
                # through the shared 1-bank pool
                units = []
                if head:
                    sa = ps_s.tile([128, 2, CS], F32, tag="s", name="s2")
                    sb = ps_s.tile([128, 2, CS], F32, tag="s", name="s2")
                    vb = ps_o.tile([128, CS], F32, tag="o", name="o_ps")
                    units.append(lambda: emit_qk_proj(cc, wq, QT2, 0,
                                                      sa[:, 0, :]))
                    units.append(lambda: emit_qk_proj(cc, wq, QT2, 1))
                    units.append(lambda: emit_qk_proj(cc, wk, KT2, 0,
                                                      sb[:, 0, :]))
                    units.append(lambda: emit_qk_proj(cc, wk, KT2, 1))
                    units.append(lambda: emit_v_tile(cc, 0, sa[:, 1, :]))
                    units.append(lambda: emit_v_tile(cc, 1, sb[:, 1, :]))
                    units.append(lambda: emit_v_tile(cc, 2, vb))
                    units.append(lambda: emit_v_tile(cc, 3))
                else:
                    units.append(lambda: emit_qk_proj(cc, wq, QT2, 0))
                    units.append(lambda: emit_qk_proj(cc, wq, QT2, 1,
                                                      defer=True))
                    units.append(lambda: emit_qk_proj(cc, wk, KT2, 0))
                    units.append(lambda: emit_qk_proj(cc, wk, KT2, 1,
                                                      defer=True))
                    for i in range(4):
                        units.append(lambda i=i: emit_v_tile(cc, i,
                                                             defer=True))
                return [(("proj", cc), 0, u) for u in units]

            # ---- global block pipeline --------------------------------
            # Score emission order: chunk 3's early C-tiles are interleaved
            # into chunk 2 (chunk 3 is exp-heavy; its scores can run early,
            # buffered in SBUF).  PV order stays natural per chunk so the
            # single-set zT accumulator never sees two chunks at once.
            pv_list = []
            for cs in range(NCS):
                for ct in range(4 * cs + 4):
                    pv_list.append((cs, ct))
            NB = len(pv_list)
            score_list = list(pv_list)
            smap = {b: k for k, b in enumerate(score_list)}

            exp_tiles = {}
            filler = []     # entries: (tag, min_block, fn)
            oc_idx = [0]
            tr_count = [0]
            drain = [False]
            cur_i = [0]

            def do_scores(blk):
                cs, ct = blk
                d = ct - 4 * cs
                off = 128 * d if d > 0 else 0
                n = CS - off
                s2 = ps_s.tile([128, 2, CS], F32, tag="s", name="s2")
                for hh in range(HPC):
                    h0 = hh * H
                    nc.tensor.matmul(
                        s2[:, hh, 0:n],
                        QT2[h0:h0 + H, ct * 128:(ct + 1) * 128],
                        KT2[h0:h0 + H, cs * CS + off:(cs + 1) * CS],
                        start=True, stop=True,
                    )
                e2 = work.tile([128, 2, CS], BF16, tag="exp",
                               bufs=ESB_BUFS, name="e2")
                nc.scalar.activation(
                    e2[:, :, 0:n], s2[:, :, 0:n],
                    mybir.ActivationFunctionType.Exp, scale=SCALE,
                )
                if d >= 0:
                    for hh in range(HPC):
                        nc.gpsimd.tensor_tensor(
                            e2[:, hh, 0:128], e2[:, hh, 0:128], mask_sb[:],
                            op=mybir.AluOpType.mult,
                        )
                exp_tiles[blk] = (e2, off, n)

            def emit_norm(cs, s):
                # recip for this sub-pair group is emitted by do_pv (it only
                # depends on the denominator columns); here: normalize both
                # heads into one [c, h2] sbuf tile, then queue transpose and
                # outproj units
                rsb = norm_recips[cs]
                zn = work.tile([128, H2], F32, tag="zn", bufs=ZN_BUFS,
                               name="zn")
                # one head normalizes on DVE, the other on ACT (activation
                # Copy with a per-partition scale AP) so the two run in
                # parallel -- this chain gates the kernel tail
                nc.vector.tensor_scalar(
                    zn[:, 0:H], ztsl(s, 0)[:, 0:H],
                    rsb[:, s * 2:s * 2 + 1], None,
                    op0=mybir.AluOpType.mult,
                )
                nc.vector.tensor_scalar(
                    zn[:, H:H2], ztsl(s, 1)[:, 0:H],
                    rsb[:, s * 2 + 1:s * 2 + 2], None,
                    op0=mybir.AluOpType.mult,
                )
                filler.insert(0, (("tr", cs), cur_i[0] + TR_DELAY,
                                  make_tr(cs, s, zn)))

            def make_tr(cs, s, zn):
                def run():
                    tr_ps = ps_o.tile([128, CS], F32, tag="o", name="o_ps")
                    nc.tensor.transpose(tr_ps[:, 0:128], zn[:], ident[:])
                    z2 = work.tile([128, 128], BF16, tag="z2", bufs=Z2_BUFS,
                                   name="z2")
                    do_copy("dve" if drain[0] else Z2C_ENGINE, z2[:],
                            tr_ps[:, 0:128])
                    o_sb = work.tile([128, E], BF16, tag="o", bufs=OSB_BUFS,
                                     name="o_sb")
                    filler.append((("out", cs), cur_i[0] + OUT_DELAY,
                                   make_out(cs, s, z2, 0, o_sb)))
                    filler.append((("out", cs), cur_i[0] + OUT_DELAY + 1,
                                   make_out(cs, s, z2, 1, o_sb)))
                return run

            def make_out(cs, s, z2, et, o_sb):
                def run():
                    last = cs == NCS - 1 and s == 3
                    if drain[0]:
                        # scores are finished: borrow the idle score banks so
                        # the final outproj chain pipelines wider
                        sl = ps_s.tile([128, 2, CS], F32, tag="s", name="s2")
                        o_ps = sl[:, et, :]
                        engs = ("act", "dve")
                    else:
                        o_ps = ps_o.tile([128, CS], F32, tag="o", name="o_ps")
                        engs = OC_ENGINES
                    nc.tensor.matmul(
                        o_ps[:], z2[:], wo[:, et * CS:(et + 1) * CS],
                        start=True, stop=True,
                    )
                    eng = engs[oc_idx[0] % len(engs)]
                    oc_idx[0] += 1
                    do_copy(eng, o_sb[:, et * CS:(et + 1) * CS], o_ps[:])
                    if last:
                        nc.sync.dma_start(
                            out_d[cs * CS + s * 128:cs * CS + (s + 1) * 128,
                                  et * CS:(et + 1) * CS],
                            o_sb[:, et * CS:(et + 1) * CS])
                    elif et == 1:
                        nc.sync.dma_start(
                            out_d[cs * CS + s * 128:cs * CS + (s + 1) * 128, :],
                            o_sb[:])
                return run

            norm_recips = {}

            def do_pv(blk):
                cs, ct = blk
                d = ct - 4 * cs
                off = 128 * d if d > 0 else 0
                diag = ct - 4 * cs  # sub index whose accumulation ends here
                # masked sub (s == d) last: its lhsT waits the DVE mask op
                subs = list(range(max(d, 0), 4))
                if d >= 0 and len(subs) > 1:
                    subs = subs[1:] + subs[:1]
                e2, off_, n = exp_tiles.pop(blk)
                bank_started = [False, False]
                for hh in range(HPC):
                    for s in subs:
                        lo = s * 128 - off
                        bank = (s * 2 + hh) // 4
                        st = False
                        if ct == 0 and not bank_started[bank]:
                            st = True
                            bank_started[bank] = True
                        nc.tensor.matmul(
                            ztsl(s, hh), e2[:, hh, lo:lo + 128],
                            V1[:, hh, ct, 0:H + 1],
                            start=st, stop=(ct == 4 * cs + s),
                            skip_group_check=True,
                        )
                if 0 <= diag < 4:
                    s = diag
                    # reciprocal granularity: subs {0,1} together, then 2, 3
                    groups = {1: (0, 1), 2: (2,), 3: (3,)}
                    if cs == 0:
                        groups = {0: (0,), 1: (1,), 2: (2,), 3: (3,)}
                    if s in groups:
                        g = groups[s]
                        lo_s, hi_s = g[0], g[-1]
                        rsb = norm_recips.get(cs)
                        if rsb is None:
                            rsb = work.tile([128, 8], F32, tag="recip",
                                            bufs=2, name="rsb")
                            norm_recips[cs] = rsb
                        for ss in g:
                            for hh in range(HPC):
                                nc.vector.reciprocal(
                                    rsb[:, ss * 2 + hh:ss * 2 + hh + 1],
                                    ztsl(ss, hh)[:, 64:65])
                        for ss in g:
                            emit_norm(cs, ss)

            def pop(nmax=1):
                npop = 0
                j = 0
                while j < len(filler) and npop < nmax:
                    tag, min_i, fn = filler[j]
                    if drain[0] or cur_i[0] >= min_i:
                        filler.pop(j)
                        fn()
                        npop += 1
                    else:
                        j += 1

            def force_proj(cs):
                j = 0
                while j < len(filler):
                    if filler[j][0] == ("proj", cs):
                        filler.pop(j)[2]()
                    else:
                        j += 1

            for i in range(NB):
                cur_i[0] = i
                cs, ct = pv_list[i]
                if ct == 0:
                    force_proj(cs)
                    if cs == 0:
                        for _, _, u in proj_units(0, head=True):
                            u()
                        filler.extend(proj_units(1))
                    elif cs == 1:
                        filler.extend(proj_units(2))
                        filler.extend(proj_units(3))
                do_scores(pv_list[i])
                npop = POPN[0]
                if len(filler) > FMAX:
                    npop += (len(filler) - FMAX + 1) // 2
                pop(npop)
                if i >= DEPTH:
                    do_pv(pv_list[i - DEPTH])
            drain[0] = True
            for i in range(NB - DEPTH, NB):
                do_pv(pv_list[i])
                pop(2)
            while filler:
                pop(1)

    with tile.TileContext(nc) as tc:
        _body(tc)
    nc.finalize()
    return nc


def _prep_inputs(x, W_Q, W_K, W_V, W_O):
    x = np.asarray(x, dtype=np.float32)
    W_Q = np.asarray(W_Q, dtype=np.float32)
    W_K = np.asarray(W_K, dtype=np.float32)
    W_V = np.asarray(W_V, dtype=np.float32)
    W_O = np.asarray(W_O, dtype=np.float32)

    xT = np.ascontiguousarray(x[0].T).astype(NPBF16)       # [E, SEQ]

    def swz(w):
        # [E, H2] -> [128(p), NKT(k), H2]
        return np.ascontiguousarray(
            w.reshape(NKT, 128, H2).transpose(1, 0, 2)).astype(NPBF16)

    in_maps = []
    for c in range(NCORES):
        a0, a1 = HPC * c, HPC * c + 1
        wq = swz(np.concatenate([W_Q[a0].T, W_Q[a1].T], axis=1))
        wk = swz(np.concatenate([W_K[a0].T, W_K[a1].T], axis=1))
        wv = swz(np.concatenate([W_V[a0].T, W_V[a1].T], axis=1))
        wo = np.ascontiguousarray(
            np.concatenate([W_O[a0].T, W_O[a1].T], axis=0)).astype(NPBF16)
        in_maps.append({"xT": xT, "wq": wq, "wk": wk, "wv": wv, "wo": wo,
                        "masksb": _MASKS, "ident": _IDENT, "onesc": _ONESC,
                        "vcol": _VCOL})
    return in_maps


_MASKS = (np.arange(128)[:, None] <= np.arange(128)[None, :]).astype(NPBF16)
_IDENT = np.eye(128, dtype=np.float32)
_ONESC = np.ones((128, 4), dtype=NPBF16)
_VCOL = np.ones((128, HPC, NCT, 2), dtype=NPBF16)


def _run(in_maps, trace=False):
    global _built
    if _built is None:
        _built = _build()
    res = bass_utils.run_bass_kernel_spmd(
        _built, in_maps, core_ids=list(range(NCORES)), trace=trace,
    )
    return res


def kernel(x, W_Q, W_K, W_V, W_O):
    in_maps = _prep_inputs(x, W_Q, W_K, W_V, W_O)
    res = _run(in_maps, trace=False)
    acc = np.zeros((SEQ, E), dtype=np.float64)
    for c in range(NCORES):
        acc += np.asarray(res.results[c]["out"], dtype=np.float64)
    return acc.astype(np.float32)[None, :, :]


def kernel_traced(x, W_Q, W_K, W_V, W_O):
    """Like kernel() but also returns a per-core exec-time estimate in ns."""
    in_maps = _prep_inputs(x, W_Q, W_K, W_V, W_O)
    exec_ns = None
    try:
        res = _run(in_maps, trace=True)
        exec_ns = res.exec_time_ns
    except Exception:
        res = _run(in_maps, trace=False)
    if exec_ns is None:
        from concourse.timeline_sim import TimelineSim
        exec_ns = int(TimelineSim(_built, trace=False).simulate())
    acc = np.zeros((SEQ, E), dtype=np.float64)
    for c in range(NCORES):
        acc += np.asarray(res.results[c]["out"], dtype=np.float64)
    return acc.astype(np.float32)[None, :, :], exec_ns


# revision 5
# speedup vs baseline: 1.0350x; 1.0337x over previous
"""Causal multi-head attention for Trainium2, head-sharded across 8 NeuronCores.

v2: transposed-PV design.  Scores are computed as ST[C, c] (C on partitions)
exactly like v1, but the PV contraction streams V as the moving operand so
z lands TRANSPOSED: zT[c, h] with the sequence position c on partitions.
That makes the softmax denominator a per-partition scalar, so normalization
is a single DVE tensor_scalar (no PE broadcast matmuls, no bc copies), and
PV matmuls stream only 64 columns per [128 c x 128 C] block instead of 512.
A cheap PE transpose per 128-row output tile restores z to [h, c] for the
output projection.

Per-core PE column budget (cycles @ 2.4 GHz):
    proj Q/K/V   49152      scores  34816      PV      17408
    denominators   ~272     transposes 2048    outproj 16384
    total ~120k cycles ~= 50 us (vs 139k = 58 us for v1).

PSUM (8 banks): merged scores 2 x [128,2,512]f32 (4 banks), zT 2 banks
([128,2,512]f32; slot = sub*2+head, 65 cols each incl. the ones-column
denominator), proj 1 bank, outproj+transpose 1 bank (WAR-chained).
start=True zeroes a whole bank, so accumulation groups never share a bank
with unrelated matmul starts.
"""

import numpy as np
import ml_dtypes

import concourse.bacc as bacc
import concourse.mybir as mybir
import concourse.tile as tile
from concourse import bass_utils

BATCH, SEQ, E, NH, H = 1, 2048, 1024, 16, 64
NCORES = 8
HPC = NH // NCORES          # heads per core
H2 = HPC * H                # 128
CS = 512                    # chunk width (c columns per score matmul)
NCS = SEQ // CS             # 4
NKT = E // 128              # 8 k-tiles over embed
NCT = SEQ // 128            # 16 C-tiles over sequence
SCALE = 1.0 / np.sqrt(H)
F32 = mybir.dt.float32
BF16 = mybir.dt.bfloat16
NPBF16 = ml_dtypes.bfloat16

_built = None

# ---- schedule knobs -------------------------------------------------------
DEPTH = 3            # PV lags scores by this many (cs, ct) blocks
ESB_BUFS = 14        # exp-output tiles in flight
ZN_BUFS = 4          # normalized zT tiles
Z2_BUFS = 4          # transposed z tiles
OSB_BUFS = 3         # outproj sbuf tiles
POPN = (2, 1)        # pop POPN[0] filler units every POPN[1] blocks
TR_SLOTS = 3         # transpose slots in the trden bank
OC_ENGINES = ("dve",)  # outproj copy engine rotation
QK_ENGINE = "dve"    # qk proj copy engine
V_ENGINE = "dve"     # v proj copy engine
Z2C_ENGINE = "dve"   # transpose psum -> sbuf copy engine
V_IN_TRDEN = False   # V-proj psum in the trden bank vs shared ps_u
INTER_LEAD = 2       # chunk-2 blocks before chunk-3 interleave starts
INTER_N = 6          # chunk-3 blocks interleaved into chunk 2
FMAX = 6             # adaptive pop: keep len(filler) near this
WARMUP = 26          # garbage matmuls before the first DMA lands (pstate ramp)
TR_DELAY = 1         # blocks between a sub's diag PV and its transpose pop
OUT_DELAY = 2        # blocks between transpose and outproj pops


def _build(stage=5):
    nc = bacc.Bacc("TRN2", target_bir_lowering=False, debug=False)

    xT_d = nc.dram_tensor("xT", [E, SEQ], BF16, kind="ExternalInput").ap()
    wq_d = nc.dram_tensor("wq", [128, NKT, H2], BF16, kind="ExternalInput").ap()
    wk_d = nc.dram_tensor("wk", [128, NKT, H2], BF16, kind="ExternalInput").ap()
    wv_d = nc.dram_tensor("wv", [128, NKT, H2], BF16, kind="ExternalInput").ap()
    wo_d = nc.dram_tensor("wo", [H2, E], BF16, kind="ExternalInput").ap()
    masks_d = nc.dram_tensor("masksb", [128, 128], BF16, kind="ExternalInput").ap()
    ident_d = nc.dram_tensor("ident", [128, 128], F32, kind="ExternalInput").ap()
    onesc_d = nc.dram_tensor("onesc", [128, 4], BF16, kind="ExternalInput").ap()
    vcol_d = nc.dram_tensor("vcol", [128, HPC, NCT, 2], BF16,
                            kind="ExternalInput").ap()
    out_d = nc.dram_tensor("out", [SEQ, E], BF16, kind="ExternalOutput").ap()

    def _body(tc):
        with (
            tc.tile_pool(name="persist", bufs=1) as persist,
            tc.tile_pool(name="work", bufs=4) as work,
            tc.tile_pool(name="ps_s", bufs=2, space="PSUM") as ps_s,
            tc.tile_pool(name="ps_zt", bufs=1, space="PSUM") as ps_zt,
            tc.tile_pool(name="ps_p", bufs=1, space="PSUM") as ps_p,
            tc.tile_pool(name="ps_o", bufs=1, space="PSUM") as ps_o,
        ):
            # ---- resident tensors -------------------------------------
            xT = persist.tile([128, NKT, SEQ], BF16)
            wq = persist.tile([128, NKT, H2], BF16)
            wk = persist.tile([128, NKT, H2], BF16)
            wv = persist.tile([128, NKT, H2], BF16)
            wo = persist.tile([128, E], BF16)
            mask_sb = persist.tile([128, 128], BF16)
            ident = persist.tile([128, 128], F32)
            onesc = persist.tile([128, 4], BF16)
            QT2 = persist.tile([128, SEQ], BF16)
            KT2 = persist.tile([128, SEQ], BF16)
            V1 = persist.tile([128, HPC, NCT, H + 2], BF16)

            # persistent PSUM accumulators (allocated once, managed by
            # subtile deps): zT [c, slot, h] and the transpose/denominator
            # bank.  slot = parity*8 + sub*2 + hh.
            # zT accumulator, 2 banks: slot idx = sub*2 + hh, bank idx//4,
            # 65 columns per slot (64 z + the ones-column denominator).
            # start=True zeroes a WHOLE psum bank, so within a PV group only
            # the first matmul touching each bank may use start=True.
            zt = ps_zt.tile([128, 2, CS], F32, name="zt")

            def ztsl(s, hh):
                idx = s * 2 + hh
                return zt[:, idx // 4, (idx % 4) * 65:(idx % 4) * 65 + 65]

            # warmup: garbage matmuls on a never-initialized tile keep the
            # PE busy during the initial DMA wait so the p-state ramp (the
            # cost model halves PE speed for the first 3us of a busy run)
            # is paid on discarded work
            if WARMUP:
                wu = persist.tile([128, 128], BF16)
                nc.gpsimd.memset(wu[:], 0.0)
                wu_ps = ps_p.tile([128, CS], F32, tag="p", name="wu_ps")
                for w in range(WARMUP):
                    nc.tensor.matmul(wu_ps[:, 0:128], wu[:], wu[:],
                                     start=True, stop=True,
                                     skip_group_check=True)

            # ---- input DMAs (issue order tracks first use) ------------
            xTv = xT_d.rearrange("(k p) c -> p k c", p=128)
            nc.sync.dma_start(wq[:, 0:2, :], wq_d[:, 0:2, :])
            nc.sync.dma_start(xT[:, 0:2, 0:CS], xTv[:, 0:2, 0:CS])
            nc.sync.dma_start(wq[:, 2:4, :], wq_d[:, 2:4, :])
            nc.sync.dma_start(xT[:, 2:4, 0:CS], xTv[:, 2:4, 0:CS])
            nc.sync.dma_start(wk[:, 0:4, :], wk_d[:, 0:4, :])
            nc.sync.dma_start(wq[:, NKT // 2:, :], wq_d[:, NKT // 2:, :])
            nc.sync.dma_start(xT[:, NKT // 2:, 0:CS], xTv[:, NKT // 2:, 0:CS])
            nc.sync.dma_start(wk[:, 4:, :], wk_d[:, 4:, :])
            nc.sync.dma_start(wv[:], wv_d[:])
            nc.sync.dma_start(mask_sb[:], masks_d[:])
            nc.sync.dma_start(ident[:], ident_d[:])
            nc.sync.dma_start(onesc[:], onesc_d[:])
            nc.sync.dma_start(V1[:, :, :, H:H + 2], vcol_d[:])
            nc.sync.dma_start(xT[:, :, CS:2 * CS], xTv[:, :, CS:2 * CS])
            nc.sync.dma_start(wo[:], wo_d[:])
            for cc in range(2, NCS):
                nc.sync.dma_start(xT[:, :, cc * CS:(cc + 1) * CS],
                                  xTv[:, :, cc * CS:(cc + 1) * CS])

            def copy_eng(which):
                return {"act": nc.scalar, "dve": nc.vector,
                        "pool": nc.gpsimd}[which]

            def do_copy(which, dst, src):
                if which == "act":
                    nc.scalar.activation(dst, src,
                                         mybir.ActivationFunctionType.Copy)
                else:
                    copy_eng(which).tensor_copy(dst, src)

            # ---- projections ------------------------------------------
            qk_live = {}

            def emit_qk_proj(cc, w_sb, dstT, half, ap_slot=None, defer=False):
                c0, c1 = cc * CS, (cc + 1) * CS
                if half == 0:
                    if ap_slot is None:
                        p_ps = ps_p.tile([128, CS], F32, tag="p", name="p_ps")
                    else:
                        p_ps = ap_slot
                    qk_live[(cc, id(dstT))] = p_ps
                else:
                    p_ps = qk_live.pop((cc, id(dstT)))
                for k in range(4 * half, 4 * half + 4):
                    nc.tensor.matmul(
                        p_ps[:], w_sb[:, k, :], xT[:, k, c0:c1],
                        start=(k == 0), stop=(k == NKT - 1),
                    )
                if half == 1:
                    # the copy pops as its own (delayed) unit so it never
                    # waits at its engine's queue head for the matmuls
                    cp = lambda: do_copy(QK_ENGINE, dstT[:, c0:c1], p_ps[:])
                    if defer:
                        filler.insert(0, (("proj", cc), cur_i[0] + 1, cp))
                    else:
                        cp()

            def emit_v_tile(cc, i, ap_slot=None, defer=False):
                ct = 4 * cc + i
                if ap_slot is not None:
                    v_ps = ap_slot
                elif V_IN_TRDEN:
                    v_ps = vregs[i % 2]
                else:
                    v_ps = ps_p.tile([128, CS], F32, tag="p", name="v_ps")
                for k in range(NKT):
                    nc.tensor.matmul(
                        v_ps[:, 0:H2], xT[:, k, ct * 128:(ct + 1) * 128],
                        wv[:, k, :],
                        start=(k == 0), stop=(k == NKT - 1),
                        skip_group_check=True,
                    )
                cp = lambda: do_copy(V_ENGINE, V1[:, :, ct, 0:H],
                                     v_ps[:, 0:H2])
                if defer:
                    filler.insert(0, (("proj", cc), cur_i[0] + 1, cp))
                else:
                    cp()

            def proj_units(cc, head=False):
                # chunk-0 projections borrow the idle score-psum halves so
                # the initial burst double-buffers; later chunks trickle
                # through the shared 1-bank pool
                units = []
                if head:
                    sa = ps_s.tile([128, 2, CS], F32, tag="s", name="s2")
                    sb = ps_s.tile([128, 2, CS], F32, tag="s", name="s2")
                    vb = ps_o.tile([128, CS], F32, tag="o", name="o_ps")
                    units.append(lambda: emit_qk_proj(cc, wq, QT2, 0,
                                                      sa[:, 0, :]))
                    units.append(lambda: emit_qk_proj(cc, wq, QT2, 1))
                    units.append(lambda: emit_qk_proj(cc, wk, KT2, 0,
                                                      sb[:, 0, :]))
                    units.append(lambda: emit_qk_proj(cc, wk, KT2, 1))
                    units.append(lambda: emit_v_tile(cc, 0, sa[:, 1, :]))
                    units.append(lambda: emit_v_tile(cc, 1, sb[:, 1, :]))
                    units.append(lambda: emit_v_tile(cc, 2, vb))
                    units.append(lambda: emit_v_tile(cc, 3))
                else:
                    units.append(lambda: emit_qk_proj(cc, wq, QT2, 0))
                    units.append(lambda: emit_qk_proj(cc, wq, QT2, 1,
                                                      defer=True))
                    units.append(lambda: emit_qk_proj(cc, wk, KT2, 0))
                    units.append(lambda: emit_qk_proj(cc, wk, KT2, 1,
                                                      defer=True))
                    for i in range(4):
                        units.append(lambda i=i: emit_v_tile(cc, i,
                                                             defer=True))
                return [(("proj", cc), 0, u) for u in units]

            # ---- global block pipeline --------------------------------
            # Score emission order: chunk 3's early C-tiles are interleaved
            # into chunk 2 (chunk 3 is exp-heavy; its scores can run early,
            # buffered in SBUF).  PV order stays natural per chunk so the
            # single-set zT accumulator never sees two chunks at once.
            pv_list = []
            for cs in range(NCS):
                for ct in range(4 * cs + 4):
                    pv_list.append((cs, ct))
            NB = len(pv_list)
            score_list = list(pv_list)
            smap = {b: k for k, b in enumerate(score_list)}

            exp_tiles = {}
            filler = []     # entries: (tag, min_block, fn)
            oc_idx = [0]
            tr_count = [0]
            drain = [False]
            cur_i = [0]

            def do_scores(blk):
                cs, ct = blk
                d = ct - 4 * cs
                off = 128 * d if d > 0 else 0
                n = CS - off
                s2 = ps_s.tile([128, 2, CS], F32, tag="s", name="s2")
                for hh in range(HPC):
                    h0 = hh * H
                    nc.tensor.matmul(
                        s2[:, hh, 0:n],
                        QT2[h0:h0 + H, ct * 128:(ct + 1) * 128],
                        KT2[h0:h0 + H, cs * CS + off:(cs + 1) * CS],
                        start=True, stop=True,
                    )
                e2 = work.tile([128, 2, CS], BF16, tag="exp",
                               bufs=ESB_BUFS, name="e2")
                nc.scalar.activation(
                    e2[:, :, 0:n], s2[:, :, 0:n],
                    mybir.ActivationFunctionType.Exp, scale=SCALE,
                )
                if d >= 0:
                    for hh in range(HPC):
                        nc.gpsimd.tensor_tensor(
                            e2[:, hh, 0:128], e2[:, hh, 0:128], mask_sb[:],
                            op=mybir.AluOpType.mult,
                        )
                exp_tiles[blk] = (e2, off, n)

            def emit_norm(cs, s):
                # recip for this sub-pair group is emitted by do_pv (it only
                # depends on the denominator columns); here: normalize both
                # heads into one [c, h2] sbuf tile, then queue transpose and
                # outproj units
                rsb = norm_recips[cs]
                zn = work.tile([128, H2], F32, tag="zn", bufs=ZN_BUFS,
                               name="zn")
                # one head normalizes on DVE, the other on ACT (activation
                # Copy with a per-partition scale AP) so the two run in
                # parallel -- this chain gates the kernel tail
                nc.vector.tensor_scalar(
                    zn[:, 0:H], ztsl(s, 0)[:, 0:H],
                    rsb[:, s * 2:s * 2 + 1], None,
                    op0=mybir.AluOpType.mult,
                )
                nc.vector.tensor_scalar(
                    zn[:, H:H2], ztsl(s, 1)[:, 0:H],
                    rsb[:, s * 2 + 1:s * 2 + 2], None,
                    op0=mybir.AluOpType.mult,
                )
                filler.insert(0, (("tr", cs), cur_i[0] + TR_DELAY,
                                  make_tr(cs, s, zn)))

            def make_tr(cs, s, zn):
                def run():
                    tr_ps = ps_o.tile([128, CS], F32, tag="o", name="o_ps")
                    nc.tensor.transpose(tr_ps[:, 0:128], zn[:], ident[:])
                    z2 = work.tile([128, 128], BF16, tag="z2", bufs=Z2_BUFS,
                                   name="z2")
                    do_copy("dve" if drain[0] else Z2C_ENGINE, z2[:],
                            tr_ps[:, 0:128])
                    o_sb = work.tile([128, E], BF16, tag="o", bufs=OSB_BUFS,
                                     name="o_sb")
                    filler.append((("out", cs), cur_i[0] + OUT_DELAY,
                                   make_out(cs, s, z2, 0, o_sb)))
                    filler.append((("out", cs), cur_i[0] + OUT_DELAY + 1,
                                   make_out(cs, s, z2, 1, o_sb)))
                return run

            def make_out(cs, s, z2, et, o_sb):
                def run():
                    last = cs == NCS - 1 and s == 3
                    if drain[0]:
                        # scores are finished: borrow the idle score banks so
                        # the final outproj chain pipelines wider
                        sl = ps_s.tile([128, 2, CS], F32, tag="s", name="s2")
                        o_ps = sl[:, et, :]
                        engs = ("act", "dve")
                    else:
                        o_ps = ps_o.tile([128, CS], F32, tag="o", name="o_ps")
                        engs = OC_ENGINES
                    nc.tensor.matmul(
                        o_ps[:], z2[:], wo[:, et * CS:(et + 1) * CS],
                        start=True, stop=True,
                    )
                    eng = engs[oc_idx[0] % len(engs)]
                    oc_idx[0] += 1
                    do_copy(eng, o_sb[:, et * CS:(et + 1) * CS], o_ps[:])
                    if last:
                        nc.sync.dma_start(
                            out_d[cs * CS + s * 128:cs * CS + (s + 1) * 128,
                                  et * CS:(et + 1) * CS],
                            o_sb[:, et * CS:(et + 1) * CS])
                    elif et == 1:
                        nc.sync.dma_start(
                            out_d[cs * CS + s * 128:cs * CS + (s + 1) * 128, :],
                            o_sb[:])
                return run

            norm_recips = {}

            def do_pv(blk):
                cs, ct = blk
                d = ct - 4 * cs
                off = 128 * d if d > 0 else 0
                diag = ct - 4 * cs  # sub index whose accumulation ends here
                # masked sub (s == d) last: its lhsT waits the DVE mask op
                subs = list(range(max(d, 0), 4))
                if d >= 0 and len(subs) > 1:
                    subs = subs[1:] + subs[:1]
                e2, off_, n = exp_tiles.pop(blk)
                bank_started = [False, False]
                for hh in range(HPC):
                    for s in subs:
                        lo = s * 128 - off
                        bank = (s * 2 + hh) // 4
                        st = False
                        if ct == 0 and not bank_started[bank]:
                            st = True
                            bank_started[bank] = True
                        nc.tensor.matmul(
                            ztsl(s, hh), e2[:, hh, lo:lo + 128],
                            V1[:, hh, ct, 0:H + 1],
                            start=st, stop=(ct == 4 * cs + s),
                            skip_group_check=True,
                        )
                if 0 <= diag < 4:
                    s = diag
                    # reciprocal granularity: subs {0,1} together, then 2, 3
                    groups = {1: (0, 1), 2: (2,), 3: (3,)}
                    if cs == 0:
                        groups = {0: (0,), 1: (1,), 2: (2,), 3: (3,)}
                    if s in groups:
                        g = groups[s]
                        lo_s, hi_s = g[0], g[-1]
                        rsb = norm_recips.get(cs)
                        if rsb is None:
                            rsb = work.tile([128, 8], F32, tag="recip",
                                            bufs=2, name="rsb")
                            norm_recips[cs] = rsb
                        for ss in g:
                            for hh in range(HPC):
                                nc.vector.reciprocal(
                                    rsb[:, ss * 2 + hh:ss * 2 + hh + 1],
                                    ztsl(ss, hh)[:, 64:65])
                        for ss in g:
                            emit_norm(cs, ss)

            def pop(nmax=1):
                npop = 0
                j = 0
                while j < len(filler) and npop < nmax:
                    tag, min_i, fn = filler[j]
                    if drain[0] or cur_i[0] >= min_i:
                        filler.pop(j)
                        fn()
                        npop += 1
                    else:
                        j += 1

            def force_proj(cs):
                j = 0
                while j < len(filler):
                    if filler[j][0] == ("proj", cs):
                        filler.pop(j)[2]()
                    else:
                        j += 1

            for i in range(NB):
                cur_i[0] = i
                cs, ct = pv_list[i]
                if ct == 0:
                    force_proj(cs)
                    if cs == 0:
                        for _, _, u in proj_units(0, head=True):
                            u()
                        filler.extend(proj_units(1))
                    elif cs == 1:
                        filler.extend(proj_units(2))
                    elif cs == 2:
                        filler.extend(proj_units(3))
                do_scores(pv_list[i])
                npop = POPN[0]
                if len(filler) > FMAX:
                    npop += (len(filler) - FMAX + 1) // 2
                pop(npop)
                if i >= DEPTH:
                    do_pv(pv_list[i - DEPTH])
            drain[0] = True
            for i in range(NB - DEPTH, NB):
                do_pv(pv_list[i])
                pop(2)
            while filler:
                pop(1)

    with tile.TileContext(nc) as tc:
        _body(tc)
    nc.finalize()
    return nc


def _prep_inputs(x, W_Q, W_K, W_V, W_O):
    x = np.asarray(x, dtype=np.float32)
    W_Q = np.asarray(W_Q, dtype=np.float32)
    W_K = np.asarray(W_K, dtype=np.float32)
    W_V = np.asarray(W_V, dtype=np.float32)
    W_O = np.asarray(W_O, dtype=np.float32)

    xT = np.ascontiguousarray(x[0].T).astype(NPBF16)       # [E, SEQ]

    def swz(w):
        # [E, H2] -> [128(p), NKT(k), H2]
        return np.ascontiguousarray(
            w.reshape(NKT, 128, H2).transpose(1, 0, 2)).astype(NPBF16)

    in_maps = []
    for c in range(NCORES):
        a0, a1 = HPC * c, HPC * c + 1
        wq = swz(np.concatenate([W_Q[a0].T, W_Q[a1].T], axis=1))
        wk = swz(np.concatenate([W_K[a0].T, W_K[a1].T], axis=1))
        wv = swz(np.concatenate([W_V[a0].T, W_V[a1].T], axis=1))
        wo = np.ascontiguousarray(
            np.concatenate([W_O[a0].T, W_O[a1].T], axis=0)).astype(NPBF16)
        in_maps.append({"xT": xT, "wq": wq, "wk": wk, "wv": wv, "wo": wo,
                        "masksb": _MASKS, "ident": _IDENT, "onesc": _ONESC,
                        "vcol": _VCOL})
    return in_maps


_MASKS = (np.arange(128)[:, None] <= np.arange(128)[None, :]).astype(NPBF16)
_IDENT = np.eye(128, dtype=np.float32)
_ONESC = np.ones((128, 4), dtype=NPBF16)
_VCOL = np.ones((128, HPC, NCT, 2), dtype=NPBF16)


def _run(in_maps, trace=False):
    global _built
    if _built is None:
        _built = _build()
    res = bass_utils.run_bass_kernel_spmd(
        _built, in_maps, core_ids=list(range(NCORES)), trace=trace,
    )
    return res


def kernel(x, W_Q, W_K, W_V, W_O):
    in_maps = _prep_inputs(x, W_Q, W_K, W_V, W_O)
    res = _run(in_maps, trace=False)
    acc = np.zeros((SEQ, E), dtype=np.float64)
    for c in range(NCORES):
        acc += np.asarray(res.results[c]["out"], dtype=np.float64)
    return acc.astype(np.float32)[None, :, :]


def kernel_traced(x, W_Q, W_K, W_V, W_O):
    """Like kernel() but also returns a per-core exec-time estimate in ns."""
    in_maps = _prep_inputs(x, W_Q, W_K, W_V, W_O)
    exec_ns = None
    try:
        res = _run(in_maps, trace=True)
        exec_ns = res.exec_time_ns
    except Exception:
        res = _run(in_maps, trace=False)
    if exec_ns is None:
        from concourse.timeline_sim import TimelineSim
        exec_ns = int(TimelineSim(_built, trace=False).simulate())
    acc = np.zeros((SEQ, E), dtype=np.float64)
    for c in range(NCORES):
        acc += np.asarray(res.results[c]["out"], dtype=np.float64)
    return acc.astype(np.float32)[None, :, :], exec_ns


# revision 6
# speedup vs baseline: 1.0442x; 1.0089x over previous
"""Causal multi-head attention for Trainium2, head-sharded across 8 NeuronCores.

v2: transposed-PV design.  Scores are computed as ST[C, c] (C on partitions)
exactly like v1, but the PV contraction streams V as the moving operand so
z lands TRANSPOSED: zT[c, h] with the sequence position c on partitions.
That makes the softmax denominator a per-partition scalar, so normalization
is a single DVE tensor_scalar (no PE broadcast matmuls, no bc copies), and
PV matmuls stream only 64 columns per [128 c x 128 C] block instead of 512.
A cheap PE transpose per 128-row output tile restores z to [h, c] for the
output projection.

Per-core PE column budget (cycles @ 2.4 GHz):
    proj Q/K/V   49152      scores  34816      PV      17408
    denominators   ~272     transposes 2048    outproj 16384
    total ~120k cycles ~= 50 us (vs 139k = 58 us for v1).

PSUM (8 banks): merged scores 2 x [128,2,512]f32 (4 banks), zT 2 banks
([128,2,512]f32; slot = sub*2+head, 65 cols each incl. the ones-column
denominator), proj 1 bank, outproj+transpose 1 bank (WAR-chained).
start=True zeroes a whole bank, so accumulation groups never share a bank
with unrelated matmul starts.
"""

import numpy as np
import ml_dtypes

import concourse.bacc as bacc
import concourse.mybir as mybir
import concourse.tile as tile
from concourse import bass_utils

BATCH, SEQ, E, NH, H = 1, 2048, 1024, 16, 64
NCORES = 8
HPC = NH // NCORES          # heads per core
H2 = HPC * H                # 128
CS = 512                    # chunk width (c columns per score matmul)
NCS = SEQ // CS             # 4
NKT = E // 128              # 8 k-tiles over embed
NCT = SEQ // 128            # 16 C-tiles over sequence
SCALE = 1.0 / np.sqrt(H)
F32 = mybir.dt.float32
BF16 = mybir.dt.bfloat16
NPBF16 = ml_dtypes.bfloat16

_built = None

# ---- schedule knobs -------------------------------------------------------
DEPTH = 3            # PV lags scores by this many (cs, ct) blocks
ESB_BUFS = 22        # exp-output tiles in flight
ZN_BUFS = 8          # normalized zT tiles
Z2_BUFS = 8          # transposed z tiles
OSB_BUFS = 5         # outproj sbuf tiles
POPN = (2, 1)        # pop POPN[0] filler units every POPN[1] blocks
TR_SLOTS = 3         # transpose slots in the trden bank
OC_ENGINES = ("dve",)  # outproj copy engine rotation
QK_ENGINE = "dve"    # qk proj copy engine
V_ENGINE = "dve"     # v proj copy engine
Z2C_ENGINE = "dve"   # transpose psum -> sbuf copy engine
V_IN_TRDEN = False   # V-proj psum in the trden bank vs shared ps_u
INTER_LEAD = 2       # chunk-2 blocks before chunk-3 interleave starts
INTER_N = 6          # chunk-3 blocks interleaved into chunk 2
FMAX = 6             # adaptive pop: keep len(filler) near this
WARMUP = 26          # garbage matmuls before the first DMA lands (pstate ramp)
TR_DELAY = 1         # blocks between a sub's diag PV and its transpose pop
OUT_DELAY = 1        # blocks between transpose and outproj pops


def _build(stage=5):
    nc = bacc.Bacc("TRN2", target_bir_lowering=False, debug=False)

    xT_d = nc.dram_tensor("xT", [E, SEQ], BF16, kind="ExternalInput").ap()
    wq_d = nc.dram_tensor("wq", [128, NKT, H2], BF16, kind="ExternalInput").ap()
    wk_d = nc.dram_tensor("wk", [128, NKT, H2], BF16, kind="ExternalInput").ap()
    wv_d = nc.dram_tensor("wv", [128, NKT, H2], BF16, kind="ExternalInput").ap()
    wo_d = nc.dram_tensor("wo", [H2, E], BF16, kind="ExternalInput").ap()
    masks_d = nc.dram_tensor("masksb", [128, 128], BF16, kind="ExternalInput").ap()
    ident_d = nc.dram_tensor("ident", [128, 128], F32, kind="ExternalInput").ap()
    onesc_d = nc.dram_tensor("onesc", [128, 4], BF16, kind="ExternalInput").ap()
    vcol_d = nc.dram_tensor("vcol", [128, HPC, NCT, 2], BF16,
                            kind="ExternalInput").ap()
    out_d = nc.dram_tensor("out", [SEQ, E], BF16, kind="ExternalOutput").ap()

    def _body(tc):
        with (
            tc.tile_pool(name="persist", bufs=1) as persist,
            tc.tile_pool(name="work", bufs=4) as work,
            tc.tile_pool(name="ps_s", bufs=2, space="PSUM") as ps_s,
            tc.tile_pool(name="ps_zt", bufs=1, space="PSUM") as ps_zt,
            tc.tile_pool(name="ps_p", bufs=1, space="PSUM") as ps_p,
            tc.tile_pool(name="ps_o", bufs=1, space="PSUM") as ps_o,
        ):
            # ---- resident tensors -------------------------------------
            xT = persist.tile([128, NKT, SEQ], BF16)
            wq = persist.tile([128, NKT, H2], BF16)
            wk = persist.tile([128, NKT, H2], BF16)
            wv = persist.tile([128, NKT, H2], BF16)
            wo = persist.tile([128, E], BF16)
            mask_sb = persist.tile([128, 128], BF16)
            ident = persist.tile([128, 128], F32)
            onesc = persist.tile([128, 4], BF16)
            QT2 = persist.tile([128, SEQ], BF16)
            KT2 = persist.tile([128, SEQ], BF16)
            V1 = persist.tile([128, HPC, NCT, H + 2], BF16)

            # persistent PSUM accumulators (allocated once, managed by
            # subtile deps): zT [c, slot, h] and the transpose/denominator
            # bank.  slot = parity*8 + sub*2 + hh.
            # zT accumulator, 2 banks: slot idx = sub*2 + hh, bank idx//4,
            # 65 columns per slot (64 z + the ones-column denominator).
            # start=True zeroes a WHOLE psum bank, so within a PV group only
            # the first matmul touching each bank may use start=True.
            zt = ps_zt.tile([128, 2, CS], F32, name="zt")

            def ztsl(s, hh):
                idx = s * 2 + hh
                return zt[:, idx // 4, (idx % 4) * 65:(idx % 4) * 65 + 65]

            # warmup: garbage matmuls on a never-initialized tile keep the
            # PE busy during the initial DMA wait so the p-state ramp (the
            # cost model halves PE speed for the first 3us of a busy run)
            # is paid on discarded work
            if WARMUP:
                wu = persist.tile([128, 128], BF16)
                nc.gpsimd.memset(wu[:], 0.0)
                wu_ps = ps_p.tile([128, CS], F32, tag="p", name="wu_ps")
                for w in range(WARMUP):
                    nc.tensor.matmul(wu_ps[:, 0:128], wu[:], wu[:],
                                     start=True, stop=True,
                                     skip_group_check=True)

            # ---- input DMAs (issue order tracks first use) ------------
            xTv = xT_d.rearrange("(k p) c -> p k c", p=128)
            nc.sync.dma_start(wq[:, 0:2, :], wq_d[:, 0:2, :])
            nc.sync.dma_start(xT[:, 0:2, 0:CS], xTv[:, 0:2, 0:CS])
            nc.sync.dma_start(wq[:, 2:4, :], wq_d[:, 2:4, :])
            nc.sync.dma_start(xT[:, 2:4, 0:CS], xTv[:, 2:4, 0:CS])
            nc.sync.dma_start(wk[:, 0:4, :], wk_d[:, 0:4, :])
            nc.sync.dma_start(wq[:, NKT // 2:, :], wq_d[:, NKT // 2:, :])
            nc.sync.dma_start(xT[:, NKT // 2:, 0:CS], xTv[:, NKT // 2:, 0:CS])
            nc.sync.dma_start(wk[:, 4:, :], wk_d[:, 4:, :])
            nc.sync.dma_start(wv[:], wv_d[:])
            nc.sync.dma_start(mask_sb[:], masks_d[:])
            nc.sync.dma_start(ident[:], ident_d[:])
            nc.sync.dma_start(onesc[:], onesc_d[:])
            nc.sync.dma_start(V1[:, :, :, H:H + 2], vcol_d[:])
            nc.sync.dma_start(xT[:, :, CS:2 * CS], xTv[:, :, CS:2 * CS])
            nc.sync.dma_start(wo[:], wo_d[:])
            for cc in range(2, NCS):
                nc.sync.dma_start(xT[:, :, cc * CS:(cc + 1) * CS],
                                  xTv[:, :, cc * CS:(cc + 1) * CS])

            def copy_eng(which):
                return {"act": nc.scalar, "dve": nc.vector,
                        "pool": nc.gpsimd}[which]

            def do_copy(which, dst, src):
                if which == "act":
                    nc.scalar.activation(dst, src,
                                         mybir.ActivationFunctionType.Copy)
                else:
                    copy_eng(which).tensor_copy(dst, src)

            # ---- projections ------------------------------------------
            qk_live = {}

            def emit_qk_proj(cc, w_sb, dstT, half, ap_slot=None, defer=False):
                c0, c1 = cc * CS, (cc + 1) * CS
                if half == 0:
                    if ap_slot is None:
                        p_ps = ps_p.tile([128, CS], F32, tag="p", name="p_ps")
                    else:
                        p_ps = ap_slot
                    qk_live[(cc, id(dstT))] = p_ps
                else:
                    p_ps = qk_live.pop((cc, id(dstT)))
                for k in range(4 * half, 4 * half + 4):
                    nc.tensor.matmul(
                        p_ps[:], w_sb[:, k, :], xT[:, k, c0:c1],
                        start=(k == 0), stop=(k == NKT - 1),
                    )
                if half == 1:
                    # the copy pops as its own (delayed) unit so it never
                    # waits at its engine's queue head for the matmuls
                    cp = lambda: do_copy(QK_ENGINE, dstT[:, c0:c1], p_ps[:])
                    if defer:
                        filler.insert(0, (("proj", cc), cur_i[0] + 1, cp))
                    else:
                        cp()

            def emit_v_tile(cc, i, ap_slot=None, defer=False):
                ct = 4 * cc + i
                if ap_slot is not None:
                    v_ps = ap_slot
                elif V_IN_TRDEN:
                    v_ps = vregs[i % 2]
                else:
                    v_ps = ps_p.tile([128, CS], F32, tag="p", name="v_ps")
                for k in range(NKT):
                    nc.tensor.matmul(
                        v_ps[:, 0:H2], xT[:, k, ct * 128:(ct + 1) * 128],
                        wv[:, k, :],
                        start=(k == 0), stop=(k == NKT - 1),
                        skip_group_check=True,
                    )
                cp = lambda: do_copy(V_ENGINE, V1[:, :, ct, 0:H],
                                     v_ps[:, 0:H2])
                if defer:
                    filler.insert(0, (("proj", cc), cur_i[0] + 1, cp))
                else:
                    cp()

            def proj_units(cc, head=False):
                # chunk-0 projections borrow the idle score-psum halves so
                # the initial burst double-buffers; later chunks trickle
                # through the shared 1-bank pool
                units = []
                if head:
                    sa = ps_s.tile([128, 2, CS], F32, tag="s", name="s2")
                    sb = ps_s.tile([128, 2, CS], F32, tag="s", name="s2")
                    vb = ps_o.tile([128, CS], F32, tag="o", name="o_ps")
                    units.append(lambda: emit_qk_proj(cc, wq, QT2, 0,
                                                      sa[:, 0, :]))
                    units.append(lambda: emit_qk_proj(cc, wq, QT2, 1))
                    units.append(lambda: emit_qk_proj(cc, wk, KT2, 0,
                                                      sb[:, 0, :]))
                    units.append(lambda: emit_qk_proj(cc, wk, KT2, 1))
                    units.append(lambda: emit_v_tile(cc, 0, sa[:, 1, :]))
                    units.append(lambda: emit_v_tile(cc, 1, sb[:, 1, :]))
                    units.append(lambda: emit_v_tile(cc, 2, vb))
                    units.append(lambda: emit_v_tile(cc, 3))
                else:
                    units.append(lambda: emit_qk_proj(cc, wq, QT2, 0))
                    units.append(lambda: emit_qk_proj(cc, wq, QT2, 1,
                                                      defer=True))
                    units.append(lambda: emit_qk_proj(cc, wk, KT2, 0))
                    units.append(lambda: emit_qk_proj(cc, wk, KT2, 1,
                                                      defer=True))
                    for i in range(4):
                        units.append(lambda i=i: emit_v_tile(cc, i,
                                                             defer=True))
                return [(("proj", cc), 0, u) for u in units]

            # ---- global block pipeline --------------------------------
            # Score emission order: chunk 3's early C-tiles are interleaved
            # into chunk 2 (chunk 3 is exp-heavy; its scores can run early,
            # buffered in SBUF).  PV order stays natural per chunk so the
            # single-set zT accumulator never sees two chunks at once.
            pv_list = []
            for cs in range(NCS):
                for ct in range(4 * cs + 4):
                    pv_list.append((cs, ct))
            NB = len(pv_list)
            score_list = list(pv_list)
            smap = {b: k for k, b in enumerate(score_list)}

            exp_tiles = {}
            filler = []     # entries: (tag, min_block, fn)
            oc_idx = [0]
            tr_count = [0]
            drain = [False]
            cur_i = [0]

            def do_scores(blk):
                cs, ct = blk
                d = ct - 4 * cs
                off = 128 * d if d > 0 else 0
                n = CS - off
                s2 = ps_s.tile([128, 2, CS], F32, tag="s", name="s2")
                for hh in range(HPC):
                    h0 = hh * H
                    nc.tensor.matmul(
                        s2[:, hh, 0:n],
                        QT2[h0:h0 + H, ct * 128:(ct + 1) * 128],
                        KT2[h0:h0 + H, cs * CS + off:(cs + 1) * CS],
                        start=True, stop=True,
                    )
                e2 = work.tile([128, 2, CS], BF16, tag="exp",
                               bufs=ESB_BUFS, name="e2")
                nc.scalar.activation(
                    e2[:, :, 0:n], s2[:, :, 0:n],
                    mybir.ActivationFunctionType.Exp, scale=SCALE,
                )
                if d >= 0:
                    for hh in range(HPC):
                        nc.gpsimd.tensor_tensor(
                            e2[:, hh, 0:128], e2[:, hh, 0:128], mask_sb[:],
                            op=mybir.AluOpType.mult,
                        )
                exp_tiles[blk] = (e2, off, n)

            def emit_norm(cs, s):
                # recip for this sub-pair group is emitted by do_pv (it only
                # depends on the denominator columns); here: normalize both
                # heads into one [c, h2] sbuf tile, then queue transpose and
                # outproj units
                rsb = norm_recips[cs]
                zn = work.tile([128, H2], F32, tag="zn", bufs=ZN_BUFS,
                               name="zn")
                # one head normalizes on DVE, the other on ACT (activation
                # Copy with a per-partition scale AP) so the two run in
                # parallel -- this chain gates the kernel tail
                nc.vector.tensor_scalar(
                    zn[:, 0:H], ztsl(s, 0)[:, 0:H],
                    rsb[:, s * 2:s * 2 + 1], None,
                    op0=mybir.AluOpType.mult,
                )
                nc.vector.tensor_scalar(
                    zn[:, H:H2], ztsl(s, 1)[:, 0:H],
                    rsb[:, s * 2 + 1:s * 2 + 2], None,
                    op0=mybir.AluOpType.mult,
                )
                filler.insert(0, (("tr", cs), cur_i[0] + TR_DELAY,
                                  make_tr(cs, s, zn)))

            def make_tr(cs, s, zn):
                def run():
                    tr_ps = ps_o.tile([128, CS], F32, tag="o", name="o_ps")
                    nc.tensor.transpose(tr_ps[:, 0:128], zn[:], ident[:])
                    z2 = work.tile([128, 128], BF16, tag="z2", bufs=Z2_BUFS,
                                   name="z2")
                    do_copy("dve" if drain[0] else Z2C_ENGINE, z2[:],
                            tr_ps[:, 0:128])
                    o_sb = work.tile([128, E], BF16, tag="o", bufs=OSB_BUFS,
                                     name="o_sb")
                    filler.append((("out", cs), cur_i[0] + OUT_DELAY,
                                   make_out(cs, s, z2, 0, o_sb)))
                    filler.append((("out", cs), cur_i[0] + OUT_DELAY + 1,
                                   make_out(cs, s, z2, 1, o_sb)))
                return run

            def make_out(cs, s, z2, et, o_sb):
                def run():
                    last = cs == NCS - 1 and s == 3
                    if drain[0]:
                        # scores are finished: borrow the idle score banks so
                        # the final outproj chain pipelines wider
                        sl = ps_s.tile([128, 2, CS], F32, tag="s", name="s2")
                        o_ps = sl[:, et, :]
                        engs = ("act", "dve")
                    else:
                        o_ps = ps_o.tile([128, CS], F32, tag="o", name="o_ps")
                        engs = OC_ENGINES
                    nc.tensor.matmul(
                        o_ps[:], z2[:], wo[:, et * CS:(et + 1) * CS],
                        start=True, stop=True,
                    )
                    eng = engs[oc_idx[0] % len(engs)]
                    oc_idx[0] += 1
                    do_copy(eng, o_sb[:, et * CS:(et + 1) * CS], o_ps[:])
                    if last:
                        nc.sync.dma_start(
                            out_d[cs * CS + s * 128:cs * CS + (s + 1) * 128,
                                  et * CS:(et + 1) * CS],
                            o_sb[:, et * CS:(et + 1) * CS])
                    elif et == 1:
                        nc.sync.dma_start(
                            out_d[cs * CS + s * 128:cs * CS + (s + 1) * 128, :],
                            o_sb[:])
                return run

            norm_recips = {}

            def do_pv(blk):
                cs, ct = blk
                d = ct - 4 * cs
                off = 128 * d if d > 0 else 0
                diag = ct - 4 * cs  # sub index whose accumulation ends here
                # masked sub (s == d) last: its lhsT waits the DVE mask op
                subs = list(range(max(d, 0), 4))
                if d >= 0 and len(subs) > 1:
                    subs = subs[1:] + subs[:1]
                e2, off_, n = exp_tiles.pop(blk)
                bank_started = [False, False]
                for hh in range(HPC):
                    for s in subs:
                        lo = s * 128 - off
                        bank = (s * 2 + hh) // 4
                        st = False
                        if ct == 0 and not bank_started[bank]:
                            st = True
                            bank_started[bank] = True
                        nc.tensor.matmul(
                            ztsl(s, hh), e2[:, hh, lo:lo + 128],
                            V1[:, hh, ct, 0:H + 1],
                            start=st, stop=(ct == 4 * cs + s),
                            skip_group_check=True,
                        )
                if 0 <= diag < 4:
                    s = diag
                    # reciprocal granularity: subs {0,1} together, then 2, 3
                    groups = {1: (0, 1), 2: (2,), 3: (3,)}
                    if cs == 0:
                        groups = {0: (0,), 1: (1,), 2: (2,), 3: (3,)}
                    if s in groups:
                        g = groups[s]
                        lo_s, hi_s = g[0], g[-1]
                        rsb = norm_recips.get(cs)
                        if rsb is None:
                            rsb = work.tile([128, 8], F32, tag="recip",
                                            bufs=2, name="rsb")
                            norm_recips[cs] = rsb
                        for ss in g:
                            for hh in range(HPC):
                                nc.vector.reciprocal(
                                    rsb[:, ss * 2 + hh:ss * 2 + hh + 1],
                                    ztsl(ss, hh)[:, 64:65])
                        for ss in g:
                            emit_norm(cs, ss)

            def pop(nmax=1):
                npop = 0
                j = 0
                while j < len(filler) and npop < nmax:
                    tag, min_i, fn = filler[j]
                    if drain[0] or cur_i[0] >= min_i:
                        filler.pop(j)
                        fn()
                        npop += 1
                    else:
                        j += 1

            def force_proj(cs):
                j = 0
                while j < len(filler):
                    if filler[j][0] == ("proj", cs):
                        filler.pop(j)[2]()
                    else:
                        j += 1

            for i in range(NB):
                cur_i[0] = i
                cs, ct = pv_list[i]
                if ct == 0:
                    force_proj(cs)
                    if cs == 0:
                        for _, _, u in proj_units(0, head=True):
                            u()
                        filler.extend(proj_units(1))
                    elif cs == 1:
                        filler.extend(proj_units(2))
                    elif cs == 2:
                        filler.extend(proj_units(3))
                do_scores(pv_list[i])
                npop = POPN[0]
                if len(filler) > FMAX:
                    npop += (len(filler) - FMAX + 1) // 2
                pop(npop)
                if i >= DEPTH:
                    do_pv(pv_list[i - DEPTH])
            drain[0] = True
            for i in range(NB - DEPTH, NB):
                do_pv(pv_list[i])
                pop(2)
            while filler:
                pop(1)

    with tile.TileContext(nc) as tc:
        _body(tc)
    nc.finalize()
    return nc


def _prep_inputs(x, W_Q, W_K, W_V, W_O):
    x = np.asarray(x, dtype=np.float32)
    W_Q = np.asarray(W_Q, dtype=np.float32)
    W_K = np.asarray(W_K, dtype=np.float32)
    W_V = np.asarray(W_V, dtype=np.float32)
    W_O = np.asarray(W_O, dtype=np.float32)

    xT = np.ascontiguousarray(x[0].T).astype(NPBF16)       # [E, SEQ]

    def swz(w):
        # [E, H2] -> [128(p), NKT(k), H2]
        return np.ascontiguousarray(
            w.reshape(NKT, 128, H2).transpose(1, 0, 2)).astype(NPBF16)

    in_maps = []
    for c in range(NCORES):
        a0, a1 = HPC * c, HPC * c + 1
        wq = swz(np.concatenate([W_Q[a0].T, W_Q[a1].T], axis=1))
        wk = swz(np.concatenate([W_K[a0].T, W_K[a1].T], axis=1))
        wv = swz(np.concatenate([W_V[a0].T, W_V[a1].T], axis=1))
        wo = np.ascontiguousarray(
            np.concatenate([W_O[a0].T, W_O[a1].T], axis=0)).astype(NPBF16)
        in_maps.append({"xT": xT, "wq": wq, "wk": wk, "wv": wv, "wo": wo,
                        "masksb": _MASKS, "ident": _IDENT, "onesc": _ONESC,
                        "vcol": _VCOL})
    return in_maps


_MASKS = (np.arange(128)[:, None] <= np.arange(128)[None, :]).astype(NPBF16)
_IDENT = np.eye(128, dtype=np.float32)
_ONESC = np.ones((128, 4), dtype=NPBF16)
_VCOL = np.ones((128, HPC, NCT, 2), dtype=NPBF16)


def _run(in_maps, trace=False):
    global _built
    if _built is None:
        _built = _build()
    res = bass_utils.run_bass_kernel_spmd(
        _built, in_maps, core_ids=list(range(NCORES)), trace=trace,
    )
    return res


def kernel(x, W_Q, W_K, W_V, W_O):
    in_maps = _prep_inputs(x, W_Q, W_K, W_V, W_O)
    res = _run(in_maps, trace=False)
    acc = np.zeros((SEQ, E), dtype=np.float64)
    for c in range(NCORES):
        acc += np.asarray(res.results[c]["out"], dtype=np.float64)
    return acc.astype(np.float32)[None, :, :]


def kernel_traced(x, W_Q, W_K, W_V, W_O):
    """Like kernel() but also returns a per-core exec-time estimate in ns."""
    in_maps = _prep_inputs(x, W_Q, W_K, W_V, W_O)
    exec_ns = None
    try:
        res = _run(in_maps, trace=True)
        exec_ns = res.exec_time_ns
    except Exception:
        res = _run(in_maps, trace=False)
    if exec_ns is None:
        from concourse.timeline_sim import TimelineSim
        exec_ns = int(TimelineSim(_built, trace=False).simulate())
    acc = np.zeros((SEQ, E), dtype=np.float64)
    for c in range(NCORES):
        acc += np.asarray(res.results[c]["out"], dtype=np.float64)
    return acc.astype(np.float32)[None, :, :], exec_ns


# revision 7
# speedup vs baseline: 1.0456x; 1.0013x over previous
"""Causal multi-head attention for Trainium2, head-sharded across 8 NeuronCores.

v2: transposed-PV design.  Scores are computed as ST[C, c] (C on partitions)
exactly like v1, but the PV contraction streams V as the moving operand so
z lands TRANSPOSED: zT[c, h] with the sequence position c on partitions.
That makes the softmax denominator a per-partition scalar, so normalization
is a single DVE tensor_scalar (no PE broadcast matmuls, no bc copies), and
PV matmuls stream only 64 columns per [128 c x 128 C] block instead of 512.
A cheap PE transpose per 128-row output tile restores z to [h, c] for the
output projection.

Per-core PE column budget (cycles @ 2.4 GHz):
    proj Q/K/V   49152      scores  34816      PV      17408
    denominators   ~272     transposes 2048    outproj 16384
    total ~120k cycles ~= 50 us (vs 139k = 58 us for v1).

PSUM (8 banks): merged scores 2 x [128,2,512]f32 (4 banks), zT 2 banks
([128,2,512]f32; slot = sub*2+head, 65 cols each incl. the ones-column
denominator), proj 1 bank, outproj+transpose 1 bank (WAR-chained).
start=True zeroes a whole bank, so accumulation groups never share a bank
with unrelated matmul starts.
"""

import numpy as np
import ml_dtypes

import concourse.bacc as bacc
import concourse.mybir as mybir
import concourse.tile as tile
from concourse import bass_utils

BATCH, SEQ, E, NH, H = 1, 2048, 1024, 16, 64
NCORES = 8
HPC = NH // NCORES          # heads per core
H2 = HPC * H                # 128
CS = 512                    # chunk width (c columns per score matmul)
NCS = SEQ // CS             # 4
NKT = E // 128              # 8 k-tiles over embed
NCT = SEQ // 128            # 16 C-tiles over sequence
SCALE = 1.0 / np.sqrt(H)
F32 = mybir.dt.float32
BF16 = mybir.dt.bfloat16
NPBF16 = ml_dtypes.bfloat16

_built = None

# ---- schedule knobs -------------------------------------------------------
DEPTH = 3            # PV lags scores by this many (cs, ct) blocks
ESB_BUFS = 22        # exp-output tiles in flight
ZN_BUFS = 8          # normalized zT tiles
Z2_BUFS = 8          # transposed z tiles
OSB_BUFS = 5         # outproj sbuf tiles
POPN = (2, 1)        # pop POPN[0] filler units every POPN[1] blocks
TR_SLOTS = 3         # transpose slots in the trden bank
OC_ENGINES = ("dve",)  # outproj copy engine rotation
QK_ENGINE = "dve"    # qk proj copy engine
V_ENGINE = "dve"     # v proj copy engine
Z2C_ENGINE = "dve"   # transpose psum -> sbuf copy engine
V_IN_TRDEN = False   # V-proj psum in the trden bank vs shared ps_u
INTER_LEAD = 2       # chunk-2 blocks before chunk-3 interleave starts
INTER_N = 6          # chunk-3 blocks interleaved into chunk 2
FMAX = 6             # adaptive pop: keep len(filler) near this
WARMUP = 26          # garbage matmuls before the first DMA lands (pstate ramp)
TR_DELAY = 1         # blocks between a sub's diag PV and its transpose pop
OUT_DELAY = 1        # blocks between transpose and outproj pops


def _build(stage=5):
    nc = bacc.Bacc("TRN2", target_bir_lowering=False, debug=False)

    xT_d = nc.dram_tensor("xT", [E, SEQ], BF16, kind="ExternalInput").ap()
    wq_d = nc.dram_tensor("wq", [128, NKT, H2], BF16, kind="ExternalInput").ap()
    wk_d = nc.dram_tensor("wk", [128, NKT, H2], BF16, kind="ExternalInput").ap()
    wv_d = nc.dram_tensor("wv", [128, NKT, H2], BF16, kind="ExternalInput").ap()
    wo_d = nc.dram_tensor("wo", [H2, E], BF16, kind="ExternalInput").ap()
    masks_d = nc.dram_tensor("masksb", [128, 128], BF16, kind="ExternalInput").ap()
    ident_d = nc.dram_tensor("ident", [128, 128], F32, kind="ExternalInput").ap()
    onesc_d = nc.dram_tensor("onesc", [128, 4], BF16, kind="ExternalInput").ap()
    vcol_d = nc.dram_tensor("vcol", [128, HPC, NCT, 2], BF16,
                            kind="ExternalInput").ap()
    out_d = nc.dram_tensor("out", [SEQ, E], BF16, kind="ExternalOutput").ap()

    def _body(tc):
        with (
            tc.tile_pool(name="persist", bufs=1) as persist,
            tc.tile_pool(name="work", bufs=4) as work,
            tc.tile_pool(name="ps_s", bufs=2, space="PSUM") as ps_s,
            tc.tile_pool(name="ps_zt", bufs=1, space="PSUM") as ps_zt,
            tc.tile_pool(name="ps_p", bufs=1, space="PSUM") as ps_p,
            tc.tile_pool(name="ps_o", bufs=1, space="PSUM") as ps_o,
        ):
            # ---- resident tensors -------------------------------------
            xT = persist.tile([128, NKT, SEQ], BF16)
            wq = persist.tile([128, NKT, H2], BF16)
            wk = persist.tile([128, NKT, H2], BF16)
            wv = persist.tile([128, NKT, H2], BF16)
            wo = persist.tile([128, E], BF16)
            mask_sb = persist.tile([128, 128], BF16)
            ident = persist.tile([128, 128], F32)
            onesc = persist.tile([128, 4], BF16)
            QT2 = persist.tile([128, SEQ], BF16)
            KT2 = persist.tile([128, SEQ], BF16)
            V1 = persist.tile([128, HPC, NCT, H + 2], BF16)

            # persistent PSUM accumulators (allocated once, managed by
            # subtile deps): zT [c, slot, h] and the transpose/denominator
            # bank.  slot = parity*8 + sub*2 + hh.
            # zT accumulator, 2 banks: slot idx = sub*2 + hh, bank idx//4,
            # 65 columns per slot (64 z + the ones-column denominator).
            # start=True zeroes a WHOLE psum bank, so within a PV group only
            # the first matmul touching each bank may use start=True.
            zt = ps_zt.tile([128, 2, CS], F32, name="zt")

            def ztsl(s, hh):
                idx = s * 2 + hh
                return zt[:, idx // 4, (idx % 4) * 65:(idx % 4) * 65 + 65]

            # warmup: garbage matmuls on a never-initialized tile keep the
            # PE busy during the initial DMA wait so the p-state ramp (the
            # cost model halves PE speed for the first 3us of a busy run)
            # is paid on discarded work
            if WARMUP:
                wu = persist.tile([128, 128], BF16)
                nc.gpsimd.memset(wu[:], 0.0)
                wu_ps = ps_p.tile([128, CS], F32, tag="p", name="wu_ps")
                for w in range(WARMUP):
                    nc.tensor.matmul(wu_ps[:, 0:128], wu[:], wu[:],
                                     start=True, stop=True,
                                     skip_group_check=True)

            # ---- input DMAs (issue order tracks first use) ------------
            xTv = xT_d.rearrange("(k p) c -> p k c", p=128)
            nc.sync.dma_start(wq[:, 0:2, :], wq_d[:, 0:2, :])
            nc.sync.dma_start(xT[:, 0:2, 0:CS], xTv[:, 0:2, 0:CS])
            nc.sync.dma_start(wq[:, 2:4, :], wq_d[:, 2:4, :])
            nc.sync.dma_start(xT[:, 2:4, 0:CS], xTv[:, 2:4, 0:CS])
            nc.sync.dma_start(wk[:, 0:4, :], wk_d[:, 0:4, :])
            nc.sync.dma_start(wq[:, NKT // 2:, :], wq_d[:, NKT // 2:, :])
            nc.sync.dma_start(xT[:, NKT // 2:, 0:CS], xTv[:, NKT // 2:, 0:CS])
            nc.sync.dma_start(wk[:, 4:, :], wk_d[:, 4:, :])
            nc.sync.dma_start(wv[:], wv_d[:])
            nc.sync.dma_start(mask_sb[:], masks_d[:])
            nc.sync.dma_start(ident[:], ident_d[:])
            nc.sync.dma_start(onesc[:], onesc_d[:])
            nc.sync.dma_start(V1[:, :, :, H:H + 2], vcol_d[:])
            nc.sync.dma_start(xT[:, :, CS:2 * CS], xTv[:, :, CS:2 * CS])
            nc.sync.dma_start(wo[:], wo_d[:])
            for cc in range(2, NCS):
                nc.sync.dma_start(xT[:, :, cc * CS:(cc + 1) * CS],
                                  xTv[:, :, cc * CS:(cc + 1) * CS])

            def copy_eng(which):
                return {"act": nc.scalar, "dve": nc.vector,
                        "pool": nc.gpsimd}[which]

            def do_copy(which, dst, src):
                if which == "act":
                    nc.scalar.activation(dst, src,
                                         mybir.ActivationFunctionType.Copy)
                else:
                    copy_eng(which).tensor_copy(dst, src)

            # ---- projections ------------------------------------------
            qk_live = {}

            def emit_qk_proj(cc, w_sb, dstT, half, ap_slot=None, defer=False):
                c0, c1 = cc * CS, (cc + 1) * CS
                if half == 0:
                    if ap_slot is None:
                        p_ps = ps_p.tile([128, CS], F32, tag="p", name="p_ps")
                    else:
                        p_ps = ap_slot
                    qk_live[(cc, id(dstT))] = p_ps
                else:
                    p_ps = qk_live.pop((cc, id(dstT)))
                for k in range(4 * half, 4 * half + 4):
                    nc.tensor.matmul(
                        p_ps[:], w_sb[:, k, :], xT[:, k, c0:c1],
                        start=(k == 0), stop=(k == NKT - 1),
                    )
                if half == 1:
                    # the copy pops as its own (delayed) unit so it never
                    # waits at its engine's queue head for the matmuls
                    cp = lambda: do_copy(QK_ENGINE, dstT[:, c0:c1], p_ps[:])
                    if defer:
                        filler.insert(0, (("proj", cc), cur_i[0] + 1, cp))
                    else:
                        cp()

            def emit_v_tile(cc, i, ap_slot=None, defer=False):
                ct = 4 * cc + i
                if ap_slot is not None:
                    v_ps = ap_slot
                elif V_IN_TRDEN:
                    v_ps = vregs[i % 2]
                else:
                    v_ps = ps_p.tile([128, CS], F32, tag="p", name="v_ps")
                for k in range(NKT):
                    nc.tensor.matmul(
                        v_ps[:, 0:H2], xT[:, k, ct * 128:(ct + 1) * 128],
                        wv[:, k, :],
                        start=(k == 0), stop=(k == NKT - 1),
                        skip_group_check=True,
                    )
                cp = lambda: do_copy(V_ENGINE, V1[:, :, ct, 0:H],
                                     v_ps[:, 0:H2])
                if defer:
                    filler.insert(0, (("proj", cc), cur_i[0] + 1, cp))
                else:
                    cp()

            def proj_units(cc, head=False):
                # chunk-0 projections borrow the idle score-psum halves so
                # the initial burst double-buffers; later chunks trickle
                # through the shared 1-bank pool
                units = []
                if head:
                    sa = ps_s.tile([128, 2, CS], F32, tag="s", name="s2")
                    sb = ps_s.tile([128, 2, CS], F32, tag="s", name="s2")
                    vb = ps_o.tile([128, CS], F32, tag="o", name="o_ps")
                    units.append(lambda: emit_qk_proj(cc, wq, QT2, 0,
                                                      sa[:, 0, :]))
                    units.append(lambda: emit_qk_proj(cc, wq, QT2, 1))
                    units.append(lambda: emit_qk_proj(cc, wk, KT2, 0,
                                                      sb[:, 0, :]))
                    units.append(lambda: emit_qk_proj(cc, wk, KT2, 1))
                    units.append(lambda: emit_v_tile(cc, 0, sa[:, 1, :]))
                    units.append(lambda: emit_v_tile(cc, 1, sb[:, 1, :]))
                    units.append(lambda: emit_v_tile(cc, 2, vb))
                    units.append(lambda: emit_v_tile(cc, 3))
                else:
                    units.append(lambda: emit_qk_proj(cc, wq, QT2, 0))
                    units.append(lambda: emit_qk_proj(cc, wq, QT2, 1,
                                                      defer=True))
                    units.append(lambda: emit_qk_proj(cc, wk, KT2, 0))
                    units.append(lambda: emit_qk_proj(cc, wk, KT2, 1,
                                                      defer=True))
                    for i in range(4):
                        units.append(lambda i=i: emit_v_tile(cc, i,
                                                             defer=True))
                return [(("proj", cc), 0, u) for u in units]

            # ---- global block pipeline --------------------------------
            # Score emission order: chunk 3's early C-tiles are interleaved
            # into chunk 2 (chunk 3 is exp-heavy; its scores can run early,
            # buffered in SBUF).  PV order stays natural per chunk so the
            # single-set zT accumulator never sees two chunks at once.
            pv_list = []
            for cs in range(NCS):
                for ct in range(4 * cs + 4):
                    pv_list.append((cs, ct))
            NB = len(pv_list)
            score_list = list(pv_list)
            smap = {b: k for k, b in enumerate(score_list)}

            exp_tiles = {}
            filler = []     # entries: (tag, min_block, fn)
            oc_idx = [0]
            tr_count = [0]
            drain = [False]
            cur_i = [0]

            def do_scores(blk):
                cs, ct = blk
                d = ct - 4 * cs
                off = 128 * d if d > 0 else 0
                n = CS - off
                s2 = ps_s.tile([128, 2, CS], F32, tag="s", name="s2")
                for hh in range(HPC):
                    h0 = hh * H
                    nc.tensor.matmul(
                        s2[:, hh, 0:n],
                        QT2[h0:h0 + H, ct * 128:(ct + 1) * 128],
                        KT2[h0:h0 + H, cs * CS + off:(cs + 1) * CS],
                        start=True, stop=True,
                    )
                e2 = work.tile([128, 2, CS], BF16, tag="exp",
                               bufs=ESB_BUFS, name="e2")
                nc.scalar.activation(
                    e2[:, :, 0:n], s2[:, :, 0:n],
                    mybir.ActivationFunctionType.Exp, scale=SCALE,
                )
                if d >= 0:
                    for hh in range(HPC):
                        nc.gpsimd.tensor_tensor(
                            e2[:, hh, 0:128], e2[:, hh, 0:128], mask_sb[:],
                            op=mybir.AluOpType.mult,
                        )
                exp_tiles[blk] = (e2, off, n)

            def emit_norm(cs, s):
                # recip for this sub-pair group is emitted by do_pv (it only
                # depends on the denominator columns); here: normalize both
                # heads into one [c, h2] sbuf tile, then queue transpose and
                # outproj units
                rsb = norm_recips[cs]
                zn = work.tile([128, H2], F32, tag="zn", bufs=ZN_BUFS,
                               name="zn")
                # one head normalizes on DVE, the other on ACT (activation
                # Copy with a per-partition scale AP) so the two run in
                # parallel -- this chain gates the kernel tail
                nc.vector.tensor_scalar(
                    zn[:, 0:H], ztsl(s, 0)[:, 0:H],
                    rsb[:, s * 2:s * 2 + 1], None,
                    op0=mybir.AluOpType.mult,
                )
                nc.vector.tensor_scalar(
                    zn[:, H:H2], ztsl(s, 1)[:, 0:H],
                    rsb[:, s * 2 + 1:s * 2 + 2], None,
                    op0=mybir.AluOpType.mult,
                )
                filler.insert(0, (("tr", cs), cur_i[0] + TR_DELAY,
                                  make_tr(cs, s, zn)))

            def make_tr(cs, s, zn):
                def run():
                    tr_ps = ps_o.tile([128, CS], F32, tag="o", name="o_ps")
                    nc.tensor.transpose(tr_ps[:, 0:128], zn[:], ident[:])
                    z2 = work.tile([128, 128], BF16, tag="z2", bufs=Z2_BUFS,
                                   name="z2")
                    do_copy("act" if cs == NCS - 1 else Z2C_ENGINE, z2[:],
                            tr_ps[:, 0:128])
                    o_sb = work.tile([128, E], BF16, tag="o", bufs=OSB_BUFS,
                                     name="o_sb")
                    filler.append((("out", cs), cur_i[0] + OUT_DELAY,
                                   make_out(cs, s, z2, 0, o_sb)))
                    filler.append((("out", cs), cur_i[0] + OUT_DELAY + 1,
                                   make_out(cs, s, z2, 1, o_sb)))
                return run

            def make_out(cs, s, z2, et, o_sb):
                def run():
                    last = cs == NCS - 1 and s == 3
                    if drain[0]:
                        # scores are finished: borrow the idle score banks so
                        # the final outproj chain pipelines wider
                        sl = ps_s.tile([128, 2, CS], F32, tag="s", name="s2")
                        o_ps = sl[:, et, :]
                        engs = ("act", "dve")
                    else:
                        o_ps = ps_o.tile([128, CS], F32, tag="o", name="o_ps")
                        engs = OC_ENGINES
                    nc.tensor.matmul(
                        o_ps[:], z2[:], wo[:, et * CS:(et + 1) * CS],
                        start=True, stop=True,
                    )
                    eng = engs[oc_idx[0] % len(engs)]
                    oc_idx[0] += 1
                    do_copy(eng, o_sb[:, et * CS:(et + 1) * CS], o_ps[:])
                    if last:
                        nc.sync.dma_start(
                            out_d[cs * CS + s * 128:cs * CS + (s + 1) * 128,
                                  et * CS:(et + 1) * CS],
                            o_sb[:, et * CS:(et + 1) * CS])
                    elif et == 1:
                        nc.sync.dma_start(
                            out_d[cs * CS + s * 128:cs * CS + (s + 1) * 128, :],
                            o_sb[:])
                return run

            norm_recips = {}

            def do_pv(blk):
                cs, ct = blk
                d = ct - 4 * cs
                off = 128 * d if d > 0 else 0
                diag = ct - 4 * cs  # sub index whose accumulation ends here
                # masked sub (s == d) last: its lhsT waits the DVE mask op
                subs = list(range(max(d, 0), 4))
                if d >= 0 and len(subs) > 1:
                    subs = subs[1:] + subs[:1]
                e2, off_, n = exp_tiles.pop(blk)
                bank_started = [False, False]
                for hh in range(HPC):
                    for s in subs:
                        lo = s * 128 - off
                        bank = (s * 2 + hh) // 4
                        st = False
                        if ct == 0 and not bank_started[bank]:
                            st = True
                            bank_started[bank] = True
                        nc.tensor.matmul(
                            ztsl(s, hh), e2[:, hh, lo:lo + 128],
                            V1[:, hh, ct, 0:H + 1],
                            start=st, stop=(ct == 4 * cs + s),
                            skip_group_check=True,
                        )
                if 0 <= diag < 4:
                    s = diag
                    # reciprocal granularity: subs {0,1} together, then 2, 3
                    groups = {1: (0, 1), 2: (2,), 3: (3,)}
                    if cs == 0:
                        groups = {0: (0,), 1: (1,), 2: (2,), 3: (3,)}
                    if s in groups:
                        g = groups[s]
                        lo_s, hi_s = g[0], g[-1]
                        rsb = norm_recips.get(cs)
                        if rsb is None:
                            rsb = work.tile([128, 8], F32, tag="recip",
                                            bufs=2, name="rsb")
                            norm_recips[cs] = rsb
                        for ss in g:
                            for hh in range(HPC):
                                nc.vector.reciprocal(
                                    rsb[:, ss * 2 + hh:ss * 2 + hh + 1],
                                    ztsl(ss, hh)[:, 64:65])
                        for ss in g:
                            emit_norm(cs, ss)

            def pop(nmax=1):
                npop = 0
                j = 0
                while j < len(filler) and npop < nmax:
                    tag, min_i, fn = filler[j]
                    if drain[0] or cur_i[0] >= min_i:
                        filler.pop(j)
                        fn()
                        npop += 1
                    else:
                        j += 1

            def force_proj(cs):
                j = 0
                while j < len(filler):
                    if filler[j][0] == ("proj", cs):
                        filler.pop(j)[2]()
                    else:
                        j += 1

            for i in range(NB):
                cur_i[0] = i
                cs, ct = pv_list[i]
                if ct == 0:
                    force_proj(cs)
                    if cs == 0:
                        for _, _, u in proj_units(0, head=True):
                            u()
                        filler.extend(proj_units(1))
                    elif cs == 1:
                        filler.extend(proj_units(2))
                    elif cs == 2:
                        filler.extend(proj_units(3))
                do_scores(pv_list[i])
                npop = POPN[0]
                if len(filler) > FMAX:
                    npop += (len(filler) - FMAX + 1) // 2
                pop(npop)
                if i >= DEPTH:
                    do_pv(pv_list[i - DEPTH])
            drain[0] = True
            for i in range(NB - DEPTH, NB):
                do_pv(pv_list[i])
                pop(2)
            while filler:
                pop(1)

    with tile.TileContext(nc) as tc:
        _body(tc)
    nc.finalize()
    return nc


def _prep_inputs(x, W_Q, W_K, W_V, W_O):
    x = np.asarray(x, dtype=np.float32)
    W_Q = np.asarray(W_Q, dtype=np.float32)
    W_K = np.asarray(W_K, dtype=np.float32)
    W_V = np.asarray(W_V, dtype=np.float32)
    W_O = np.asarray(W_O, dtype=np.float32)

    xT = np.ascontiguousarray(x[0].T).astype(NPBF16)       # [E, SEQ]

    def swz(w):
        # [E, H2] -> [128(p), NKT(k), H2]
        return np.ascontiguousarray(
            w.reshape(NKT, 128, H2).transpose(1, 0, 2)).astype(NPBF16)

    in_maps = []
    for c in range(NCORES):
        a0, a1 = HPC * c, HPC * c + 1
        wq = swz(np.concatenate([W_Q[a0].T, W_Q[a1].T], axis=1))
        wk = swz(np.concatenate([W_K[a0].T, W_K[a1].T], axis=1))
        wv = swz(np.concatenate([W_V[a0].T, W_V[a1].T], axis=1))
        wo = np.ascontiguousarray(
            np.concatenate([W_O[a0].T, W_O[a1].T], axis=0)).astype(NPBF16)
        in_maps.append({"xT": xT, "wq": wq, "wk": wk, "wv": wv, "wo": wo,
                        "masksb": _MASKS, "ident": _IDENT, "onesc": _ONESC,
                        "vcol": _VCOL})
    return in_maps


_MASKS = (np.arange(128)[:, None] <= np.arange(128)[None, :]).astype(NPBF16)
_IDENT = np.eye(128, dtype=np.float32)
_ONESC = np.ones((128, 4), dtype=NPBF16)
_VCOL = np.ones((128, HPC, NCT, 2), dtype=NPBF16)


def _run(in_maps, trace=False):
    global _built
    if _built is None:
        _built = _build()
    res = bass_utils.run_bass_kernel_spmd(
        _built, in_maps, core_ids=list(range(NCORES)), trace=trace,
    )
    return res


def kernel(x, W_Q, W_K, W_V, W_O):
    in_maps = _prep_inputs(x, W_Q, W_K, W_V, W_O)
    res = _run(in_maps, trace=False)
    acc = np.zeros((SEQ, E), dtype=np.float64)
    for c in range(NCORES):
        acc += np.asarray(res.results[c]["out"], dtype=np.float64)
    return acc.astype(np.float32)[None, :, :]


def kernel_traced(x, W_Q, W_K, W_V, W_O):
    """Like kernel() but also returns a per-core exec-time estimate in ns."""
    in_maps = _prep_inputs(x, W_Q, W_K, W_V, W_O)
    exec_ns = None
    try:
        res = _run(in_maps, trace=True)
        exec_ns = res.exec_time_ns
    except Exception:
        res = _run(in_maps, trace=False)
    if exec_ns is None:
        from concourse.timeline_sim import TimelineSim
        exec_ns = int(TimelineSim(_built, trace=False).simulate())
    acc = np.zeros((SEQ, E), dtype=np.float64)
    for c in range(NCORES):
        acc += np.asarray(res.results[c]["out"], dtype=np.float64)
    return acc.astype(np.float32)[None, :, :], exec_ns


# revision 8
# speedup vs baseline: 1.0883x; 1.0409x over previous
"""Causal multi-head attention for Trainium2, head-sharded across 8 NeuronCores.

v2: transposed-PV design.  Scores are computed as ST[C, c] (C on partitions)
exactly like v1, but the PV contraction streams V as the moving operand so
z lands TRANSPOSED: zT[c, h] with the sequence position c on partitions.
That makes the softmax denominator a per-partition scalar, so normalization
is a single DVE tensor_scalar (no PE broadcast matmuls, no bc copies), and
PV matmuls stream only 64 columns per [128 c x 128 C] block instead of 512.
A cheap PE transpose per 128-row output tile restores z to [h, c] for the
output projection.

Per-core PE column budget (cycles @ 2.4 GHz):
    proj Q/K/V   49152      scores  34816      PV      17408
    denominators   ~272     transposes 2048    outproj 16384
    total ~120k cycles ~= 50 us (vs 139k = 58 us for v1).

PSUM (8 banks): merged scores 2 x [128,2,512]f32 (4 banks), zT 2 banks
([128,2,512]f32; slot = sub*2+head, 65 cols each incl. the ones-column
denominator), proj 1 bank, outproj+transpose 1 bank (WAR-chained).
start=True zeroes a whole bank, so accumulation groups never share a bank
with unrelated matmul starts.
"""

import numpy as np
import ml_dtypes

import concourse.bacc as bacc
import concourse.mybir as mybir
import concourse.tile as tile
from concourse import bass_utils

BATCH, SEQ, E, NH, H = 1, 2048, 1024, 16, 64
NCORES = 8
HPC = NH // NCORES          # heads per core
H2 = HPC * H                # 128
CS = 512                    # chunk width (c columns per score matmul)
NCS = SEQ // CS             # 4
NKT = E // 128              # 8 k-tiles over embed
NCT = SEQ // 128            # 16 C-tiles over sequence
SCALE = 1.0 / np.sqrt(H)
F32 = mybir.dt.float32
BF16 = mybir.dt.bfloat16
NPBF16 = ml_dtypes.bfloat16

_built = None

# ---- schedule knobs -------------------------------------------------------
DEPTH = 3            # PV lags scores by this many (cs, ct) blocks
ESB_BUFS = 22        # exp-output tiles in flight
ZN_BUFS = 8          # normalized zT tiles
Z2_BUFS = 8          # transposed z tiles
OSB_BUFS = 5         # outproj sbuf tiles
POPN = (2, 1)        # pop POPN[0] filler units every POPN[1] blocks
TR_SLOTS = 3         # transpose slots in the trden bank
OC_ENGINES = ("dve",)  # outproj copy engine rotation
QK_ENGINE = "dve"    # qk proj copy engine
V_ENGINE = "dve"     # v proj copy engine
Z2C_ENGINE = "dve"   # transpose psum -> sbuf copy engine
V_IN_TRDEN = False   # V-proj psum in the trden bank vs shared ps_u
INTER_LEAD = 2       # chunk-2 blocks before chunk-3 interleave starts
INTER_N = 6          # chunk-3 blocks interleaved into chunk 2
FMAX = 6             # adaptive pop: keep len(filler) near this
WARMUP = 26          # garbage matmuls before the first DMA lands (pstate ramp)
TR_DELAY = 1         # blocks between a sub's diag PV and its transpose pop
OUT_DELAY = 1        # blocks between transpose and outproj pops


def _build(stage=5):
    nc = bacc.Bacc("TRN2", target_bir_lowering=False, debug=False)

    xT_d = nc.dram_tensor("xT", [E, SEQ], BF16, kind="ExternalInput").ap()
    wq_d = nc.dram_tensor("wq", [128, NKT, H2], BF16, kind="ExternalInput").ap()
    wk_d = nc.dram_tensor("wk", [128, NKT, H2], BF16, kind="ExternalInput").ap()
    wv_d = nc.dram_tensor("wv", [128, NKT, H2], BF16, kind="ExternalInput").ap()
    wo_d = nc.dram_tensor("wo", [H2, E], BF16, kind="ExternalInput").ap()
    masks_d = nc.dram_tensor("masksb", [128, 128], BF16, kind="ExternalInput").ap()
    ident_d = nc.dram_tensor("ident", [128, 128], F32, kind="ExternalInput").ap()
    vcol_d = nc.dram_tensor("vcol", [128, HPC, NCT, 2], BF16,
                            kind="ExternalInput").ap()
    out_d = nc.dram_tensor("out", [SEQ, E], BF16, kind="ExternalOutput").ap()

    def _body(tc):
        with (
            tc.tile_pool(name="persist", bufs=1) as persist,
            tc.tile_pool(name="work", bufs=4) as work,
            tc.tile_pool(name="ps_s", bufs=2, space="PSUM") as ps_s,
            tc.tile_pool(name="ps_zt", bufs=1, space="PSUM") as ps_zt,
            tc.tile_pool(name="ps_p", bufs=1, space="PSUM") as ps_p,
            tc.tile_pool(name="ps_o", bufs=1, space="PSUM") as ps_o,
        ):
            # ---- resident tensors -------------------------------------
            xT = persist.tile([128, NKT, SEQ], BF16)
            wq = persist.tile([128, NKT, H2], BF16)
            wk = persist.tile([128, NKT, H2], BF16)
            wv = persist.tile([128, NKT, H2], BF16)
            wo = persist.tile([128, E], BF16)
            mask_sb = persist.tile([128, 128], BF16)
            ident = persist.tile([128, 128], F32)
            QT2 = persist.tile([128, SEQ], BF16)
            KT2 = persist.tile([128, SEQ], BF16)
            V1 = persist.tile([128, HPC, NCT, H + 2], BF16)

            # persistent PSUM accumulators (allocated once, managed by
            # subtile deps): zT [c, slot, h] and the transpose/denominator
            # bank.  slot = parity*8 + sub*2 + hh.
            # zT accumulator, 2 banks: slot idx = sub*2 + hh, bank idx//4,
            # 65 columns per slot (64 z + the ones-column denominator).
            # start=True zeroes a WHOLE psum bank, so within a PV group only
            # the first matmul touching each bank may use start=True.
            zt = ps_zt.tile([128, 2, CS], F32, name="zt")

            def ztsl(s, hh):
                idx = s * 2 + hh
                return zt[:, idx // 4, (idx % 4) * 65:(idx % 4) * 65 + 65]

            # warmup: garbage matmuls on a never-initialized tile keep the
            # PE busy during the initial DMA wait so the p-state ramp (the
            # cost model halves PE speed for the first 3us of a busy run)
            # is paid on discarded work
            if WARMUP:
                wu = persist.tile([128, 128], BF16)
                nc.gpsimd.memset(wu[:], 0.0)
                wu_ps = ps_p.tile([128, CS], F32, tag="p", name="wu_ps")
                for w in range(WARMUP):
                    nc.tensor.matmul(wu_ps[:, 0:128], wu[:], wu[:],
                                     start=True, stop=True,
                                     skip_group_check=True)

            # ---- input DMAs (issue order tracks first use) ------------
            xTv = xT_d.rearrange("(k p) c -> p k c", p=128)
            nc.sync.dma_start(wq[:, 0:2, :], wq_d[:, 0:2, :])
            nc.sync.dma_start(xT[:, 0:2, 0:CS], xTv[:, 0:2, 0:CS])
            nc.sync.dma_start(wq[:, 2:4, :], wq_d[:, 2:4, :])
            nc.sync.dma_start(xT[:, 2:4, 0:CS], xTv[:, 2:4, 0:CS])
            nc.sync.dma_start(wk[:, 0:4, :], wk_d[:, 0:4, :])
            nc.sync.dma_start(wq[:, NKT // 2:, :], wq_d[:, NKT // 2:, :])
            nc.sync.dma_start(xT[:, NKT // 2:, 0:CS], xTv[:, NKT // 2:, 0:CS])
            nc.sync.dma_start(wk[:, 4:, :], wk_d[:, 4:, :])
            nc.sync.dma_start(wv[:], wv_d[:])
            nc.sync.dma_start(mask_sb[:], masks_d[:])
            nc.sync.dma_start(xT[:, :, CS:2 * CS], xTv[:, :, CS:2 * CS])
            nc.sync.dma_start(V1[:, :, :, H:H + 2], vcol_d[:])
            nc.sync.dma_start(ident[:], ident_d[:])
            nc.sync.dma_start(wo[:], wo_d[:])
            for cc in range(2, NCS):
                nc.sync.dma_start(xT[:, :, cc * CS:(cc + 1) * CS],
                                  xTv[:, :, cc * CS:(cc + 1) * CS])

            def copy_eng(which):
                return {"act": nc.scalar, "dve": nc.vector,
                        "pool": nc.gpsimd}[which]

            def do_copy(which, dst, src):
                if which == "act":
                    nc.scalar.activation(dst, src,
                                         mybir.ActivationFunctionType.Copy)
                else:
                    copy_eng(which).tensor_copy(dst, src)

            # ---- projections ------------------------------------------
            qk_live = {}

            def emit_qk_proj(cc, w_sb, dstT, half, ap_slot=None, defer=False):
                c0, c1 = cc * CS, (cc + 1) * CS
                if half == 0:
                    if ap_slot is None:
                        p_ps = ps_p.tile([128, CS], F32, tag="p", name="p_ps")
                    else:
                        p_ps = ap_slot
                    qk_live[(cc, id(dstT))] = p_ps
                else:
                    p_ps = qk_live.pop((cc, id(dstT)))
                for k in range(4 * half, 4 * half + 4):
                    nc.tensor.matmul(
                        p_ps[:], w_sb[:, k, :], xT[:, k, c0:c1],
                        start=(k == 0), stop=(k == NKT - 1),
                    )
                if half == 1:
                    # the copy pops as its own (delayed) unit so it never
                    # waits at its engine's queue head for the matmuls
                    cp = lambda: do_copy(QK_ENGINE, dstT[:, c0:c1], p_ps[:])
                    if defer:
                        filler.insert(0, (("proj", cc), cur_i[0] + 1, cp))
                    else:
                        cp()

            def emit_v_tile(cc, i, ap_slot=None, defer=False):
                ct = 4 * cc + i
                if ap_slot is not None:
                    v_ps = ap_slot
                elif V_IN_TRDEN:
                    v_ps = vregs[i % 2]
                else:
                    v_ps = ps_p.tile([128, CS], F32, tag="p", name="v_ps")
                for k in range(NKT):
                    nc.tensor.matmul(
                        v_ps[:, 0:H2], xT[:, k, ct * 128:(ct + 1) * 128],
                        wv[:, k, :],
                        start=(k == 0), stop=(k == NKT - 1),
                        skip_group_check=True,
                    )
                cp = lambda: do_copy(V_ENGINE, V1[:, :, ct, 0:H],
                                     v_ps[:, 0:H2])
                if defer:
                    filler.insert(0, (("proj", cc), cur_i[0] + 1, cp))
                else:
                    cp()

            def proj_units(cc, head=False):
                # chunk-0 projections borrow the idle score-psum halves so
                # the initial burst double-buffers; later chunks trickle
                # through the shared 1-bank pool
                units = []
                if head:
                    sa = ps_s.tile([128, 2, CS], F32, tag="s", name="s2")
                    sb = ps_s.tile([128, 2, CS], F32, tag="s", name="s2")
                    vb = ps_o.tile([128, CS], F32, tag="o", name="o_ps")
                    units.append(lambda: emit_qk_proj(cc, wq, QT2, 0,
                                                      sa[:, 0, :]))
                    units.append(lambda: emit_qk_proj(cc, wq, QT2, 1))
                    units.append(lambda: emit_qk_proj(cc, wk, KT2, 0,
                                                      sb[:, 0, :]))
                    units.append(lambda: emit_qk_proj(cc, wk, KT2, 1))
                    units.append(lambda: emit_v_tile(cc, 0, sa[:, 1, :]))
                    units.append(lambda: emit_v_tile(cc, 1, sb[:, 1, :]))
                    units.append(lambda: emit_v_tile(cc, 2, vb))
                    units.append(lambda: emit_v_tile(cc, 3))
                else:
                    units.append(lambda: emit_qk_proj(cc, wq, QT2, 0))
                    units.append(lambda: emit_qk_proj(cc, wq, QT2, 1,
                                                      defer=True))
                    units.append(lambda: emit_qk_proj(cc, wk, KT2, 0))
                    units.append(lambda: emit_qk_proj(cc, wk, KT2, 1,
                                                      defer=True))
                    for i in range(4):
                        units.append(lambda i=i: emit_v_tile(cc, i,
                                                             defer=True))
                return [(("proj", cc), 0, u) for u in units]

            # ---- global block pipeline --------------------------------
            # Score emission order: chunk 3's early C-tiles are interleaved
            # into chunk 2 (chunk 3 is exp-heavy; its scores can run early,
            # buffered in SBUF).  PV order stays natural per chunk so the
            # single-set zT accumulator never sees two chunks at once.
            pv_list = []
            for cs in range(NCS):
                for ct in range(4 * cs + 4):
                    pv_list.append((cs, ct))
            NB = len(pv_list)
            score_list = list(pv_list)
            smap = {b: k for k, b in enumerate(score_list)}

            exp_tiles = {}
            filler = []     # entries: (tag, min_block, fn)
            oc_idx = [0]
            tr_count = [0]
            drain = [False]
            cur_i = [0]

            def do_scores(blk):
                cs, ct = blk
                d = ct - 4 * cs
                off = 128 * d if d > 0 else 0
                n = CS - off
                s2 = ps_s.tile([128, 2, CS], F32, tag="s", name="s2")
                for hh in range(HPC):
                    h0 = hh * H
                    nc.tensor.matmul(
                        s2[:, hh, 0:n],
                        QT2[h0:h0 + H, ct * 128:(ct + 1) * 128],
                        KT2[h0:h0 + H, cs * CS + off:(cs + 1) * CS],
                        start=True, stop=True,
                    )
                e2 = work.tile([128, 2, CS], BF16, tag="exp",
                               bufs=ESB_BUFS, name="e2")
                nc.scalar.activation(
                    e2[:, :, 0:n], s2[:, :, 0:n],
                    mybir.ActivationFunctionType.Exp, scale=SCALE,
                )
                if d >= 0:
                    for hh in range(HPC):
                        nc.gpsimd.tensor_tensor(
                            e2[:, hh, 0:128], e2[:, hh, 0:128], mask_sb[:],
                            op=mybir.AluOpType.mult,
                        )
                exp_tiles[blk] = (e2, off, n)

            def emit_norm(cs, s):
                # recip for this sub-pair group is emitted by do_pv (it only
                # depends on the denominator columns); here: normalize both
                # heads into one [c, h2] sbuf tile, then queue transpose and
                # outproj units
                rsb = norm_recips[cs]
                zn = work.tile([128, H2], F32, tag="zn", bufs=ZN_BUFS,
                               name="zn")
                # one head normalizes on DVE, the other on ACT (activation
                # Copy with a per-partition scale AP) so the two run in
                # parallel -- this chain gates the kernel tail
                nc.vector.tensor_scalar(
                    zn[:, 0:H], ztsl(s, 0)[:, 0:H],
                    rsb[:, s * 2:s * 2 + 1], None,
                    op0=mybir.AluOpType.mult,
                )
                nc.vector.tensor_scalar(
                    zn[:, H:H2], ztsl(s, 1)[:, 0:H],
                    rsb[:, s * 2 + 1:s * 2 + 2], None,
                    op0=mybir.AluOpType.mult,
                )
                filler.insert(0, (("tr", cs), cur_i[0] + TR_DELAY,
                                  make_tr(cs, s, zn)))

            def make_tr(cs, s, zn):
                def run():
                    tr_ps = ps_o.tile([128, CS], F32, tag="o", name="o_ps")
                    nc.tensor.transpose(tr_ps[:, 0:128], zn[:], ident[:])
                    z2 = work.tile([128, 128], BF16, tag="z2", bufs=Z2_BUFS,
                                   name="z2")
                    do_copy("act" if cs == NCS - 1 else Z2C_ENGINE, z2[:],
                            tr_ps[:, 0:128])
                    o_sb = work.tile([128, E], BF16, tag="o", bufs=OSB_BUFS,
                                     name="o_sb")
                    filler.append((("out", cs), cur_i[0] + OUT_DELAY,
                                   make_out(cs, s, z2, 0, o_sb)))
                    filler.append((("out", cs), cur_i[0] + OUT_DELAY + 1,
                                   make_out(cs, s, z2, 1, o_sb)))
                return run

            def make_out(cs, s, z2, et, o_sb):
                def run():
                    last = cs == NCS - 1 and s == 3
                    if drain[0]:
                        # scores are finished: borrow the idle score banks so
                        # the final outproj chain pipelines wider
                        sl = ps_s.tile([128, 2, CS], F32, tag="s", name="s2")
                        o_ps = sl[:, et, :]
                        engs = ("act", "dve")
                    else:
                        o_ps = ps_o.tile([128, CS], F32, tag="o", name="o_ps")
                        engs = OC_ENGINES
                    nc.tensor.matmul(
                        o_ps[:], z2[:], wo[:, et * CS:(et + 1) * CS],
                        start=True, stop=True,
                    )
                    eng = engs[oc_idx[0] % len(engs)]
                    oc_idx[0] += 1
                    do_copy(eng, o_sb[:, et * CS:(et + 1) * CS], o_ps[:])
                    if last:
                        nc.sync.dma_start(
                            out_d[cs * CS + s * 128:cs * CS + (s + 1) * 128,
                                  et * CS:(et + 1) * CS],
                            o_sb[:, et * CS:(et + 1) * CS])
                    elif et == 1:
                        nc.sync.dma_start(
                            out_d[cs * CS + s * 128:cs * CS + (s + 1) * 128, :],
                            o_sb[:])
                return run

            norm_recips = {}

            def do_pv(blk):
                cs, ct = blk
                d = ct - 4 * cs
                off = 128 * d if d > 0 else 0
                diag = ct - 4 * cs  # sub index whose accumulation ends here
                # masked sub (s == d) last: its lhsT waits the DVE mask op
                subs = list(range(max(d, 0), 4))
                if d >= 0 and len(subs) > 1:
                    subs = subs[1:] + subs[:1]
                e2, off_, n = exp_tiles.pop(blk)
                bank_started = [False, False]
                for hh in range(HPC):
                    for s in subs:
                        lo = s * 128 - off
                        bank = (s * 2 + hh) // 4
                        st = False
                        if ct == 0 and not bank_started[bank]:
                            st = True
                            bank_started[bank] = True
                        nc.tensor.matmul(
                            ztsl(s, hh), e2[:, hh, lo:lo + 128],
                            V1[:, hh, ct, 0:H + 1],
                            start=st, stop=(ct == 4 * cs + s),
                            skip_group_check=True,
                        )
                if 0 <= diag < 4:
                    s = diag
                    # reciprocal granularity: subs {0,1} together, then 2, 3
                    groups = {1: (0, 1), 2: (2,), 3: (3,)}
                    if cs == 0:
                        groups = {0: (0,), 1: (1,), 2: (2,), 3: (3,)}
                    if s in groups:
                        g = groups[s]
                        lo_s, hi_s = g[0], g[-1]
                        rsb = norm_recips.get(cs)
                        if rsb is None:
                            rsb = work.tile([128, 8], F32, tag="recip",
                                            bufs=2, name="rsb")
                            norm_recips[cs] = rsb
                        for ss in g:
                            for hh in range(HPC):
                                nc.vector.reciprocal(
                                    rsb[:, ss * 2 + hh:ss * 2 + hh + 1],
                                    ztsl(ss, hh)[:, 64:65])
                        for ss in g:
                            emit_norm(cs, ss)

            def pop(nmax=1):
                npop = 0
                j = 0
                while j < len(filler) and npop < nmax:
                    tag, min_i, fn = filler[j]
                    if drain[0] or cur_i[0] >= min_i:
                        filler.pop(j)
                        fn()
                        npop += 1
                    else:
                        j += 1

            def force_proj(cs):
                j = 0
                while j < len(filler):
                    if filler[j][0] == ("proj", cs):
                        filler.pop(j)[2]()
                    else:
                        j += 1

            for i in range(NB):
                cur_i[0] = i
                cs, ct = pv_list[i]
                if ct == 0:
                    force_proj(cs)
                    if cs == 0:
                        for _, _, u in proj_units(0, head=True):
                            u()
                        filler.extend(proj_units(1))
                    elif cs == 1:
                        filler.extend(proj_units(2))
                    elif cs == 2:
                        filler.extend(proj_units(3))
                do_scores(pv_list[i])
                npop = POPN[0]
                if len(filler) > FMAX:
                    npop += (len(filler) - FMAX + 1) // 2
                pop(npop)
                if i >= DEPTH:
                    do_pv(pv_list[i - DEPTH])
            drain[0] = True
            for i in range(NB - DEPTH, NB):
                do_pv(pv_list[i])
                pop(2)
            while filler:
                pop(1)

    with tile.TileContext(nc) as tc:
        _body(tc)
    nc.finalize()
    return nc


def _prep_inputs(x, W_Q, W_K, W_V, W_O):
    x = np.asarray(x, dtype=np.float32)
    W_Q = np.asarray(W_Q, dtype=np.float32)
    W_K = np.asarray(W_K, dtype=np.float32)
    W_V = np.asarray(W_V, dtype=np.float32)
    W_O = np.asarray(W_O, dtype=np.float32)

    xT = np.ascontiguousarray(x[0].T).astype(NPBF16)       # [E, SEQ]

    def swz(w):
        # [E, H2] -> [128(p), NKT(k), H2]
        return np.ascontiguousarray(
            w.reshape(NKT, 128, H2).transpose(1, 0, 2)).astype(NPBF16)

    in_maps = []
    for c in range(NCORES):
        a0, a1 = HPC * c, HPC * c + 1
        wq = swz(np.concatenate([W_Q[a0].T, W_Q[a1].T], axis=1))
        wk = swz(np.concatenate([W_K[a0].T, W_K[a1].T], axis=1))
        wv = swz(np.concatenate([W_V[a0].T, W_V[a1].T], axis=1))
        wo = np.ascontiguousarray(
            np.concatenate([W_O[a0].T, W_O[a1].T], axis=0)).astype(NPBF16)
        in_maps.append({"xT": xT, "wq": wq, "wk": wk, "wv": wv, "wo": wo,
                        "masksb": _MASKS, "ident": _IDENT, "vcol": _VCOL})
    return in_maps


_MASKS = (np.arange(128)[:, None] <= np.arange(128)[None, :]).astype(NPBF16)
_IDENT = np.eye(128, dtype=np.float32)
_VCOL = np.ones((128, HPC, NCT, 2), dtype=NPBF16)


def _run(in_maps, trace=False):
    global _built
    if _built is None:
        _built = _build()
    res = bass_utils.run_bass_kernel_spmd(
        _built, in_maps, core_ids=list(range(NCORES)), trace=trace,
    )
    return res


def kernel(x, W_Q, W_K, W_V, W_O):
    in_maps = _prep_inputs(x, W_Q, W_K, W_V, W_O)
    res = _run(in_maps, trace=False)
    acc = np.zeros((SEQ, E), dtype=np.float64)
    for c in range(NCORES):
        acc += np.asarray(res.results[c]["out"], dtype=np.float64)
    return acc.astype(np.float32)[None, :, :]


def kernel_traced(x, W_Q, W_K, W_V, W_O):
    """Like kernel() but also returns a per-core exec-time estimate in ns."""
    in_maps = _prep_inputs(x, W_Q, W_K, W_V, W_O)
    exec_ns = None
    try:
        res = _run(in_maps, trace=True)
        exec_ns = res.exec_time_ns
    except Exception:
        res = _run(in_maps, trace=False)
    if exec_ns is None:
        from concourse.timeline_sim import TimelineSim
        exec_ns = int(TimelineSim(_built, trace=False).simulate())
    acc = np.zeros((SEQ, E), dtype=np.float64)
    for c in range(NCORES):
        acc += np.asarray(res.results[c]["out"], dtype=np.float64)
    return acc.astype(np.float32)[None, :, :], exec_ns


# revision 9
# speedup vs baseline: 1.0951x; 1.0062x over previous
"""Causal multi-head attention for Trainium2, head-sharded across 8 NeuronCores.

v2: transposed-PV design.  Scores are computed as ST[C, c] (C on partitions)
exactly like v1, but the PV contraction streams V as the moving operand so
z lands TRANSPOSED: zT[c, h] with the sequence position c on partitions.
That makes the softmax denominator a per-partition scalar, so normalization
is a single DVE tensor_scalar (no PE broadcast matmuls, no bc copies), and
PV matmuls stream only 64 columns per [128 c x 128 C] block instead of 512.
A cheap PE transpose per 128-row output tile restores z to [h, c] for the
output projection.

Per-core PE column budget (cycles @ 2.4 GHz):
    proj Q/K/V   49152      scores  34816      PV      17408
    denominators   ~272     transposes 2048    outproj 16384
    total ~120k cycles ~= 50 us (vs 139k = 58 us for v1).

PSUM (8 banks): merged scores 2 x [128,2,512]f32 (4 banks), zT 2 banks
([128,2,512]f32; slot = sub*2+head, 65 cols each incl. the ones-column
denominator), proj 1 bank, outproj+transpose 1 bank (WAR-chained).
start=True zeroes a whole bank, so accumulation groups never share a bank
with unrelated matmul starts.
"""

import numpy as np
import ml_dtypes

import concourse.bacc as bacc
import concourse.mybir as mybir
import concourse.tile as tile
from concourse import bass_utils

BATCH, SEQ, E, NH, H = 1, 2048, 1024, 16, 64
NCORES = 8
HPC = NH // NCORES          # heads per core
H2 = HPC * H                # 128
CS = 512                    # chunk width (c columns per score matmul)
NCS = SEQ // CS             # 4
NKT = E // 128              # 8 k-tiles over embed
NCT = SEQ // 128            # 16 C-tiles over sequence
SCALE = 1.0 / np.sqrt(H)
F32 = mybir.dt.float32
BF16 = mybir.dt.bfloat16
NPBF16 = ml_dtypes.bfloat16

_built = None

# ---- schedule knobs -------------------------------------------------------
DEPTH = 3            # PV lags scores by this many (cs, ct) blocks
ESB_BUFS = 22        # exp-output tiles in flight
ZN_BUFS = 8          # normalized zT tiles
Z2_BUFS = 8          # transposed z tiles
OSB_BUFS = 5         # outproj sbuf tiles
POPN = (2, 1)        # pop POPN[0] filler units every POPN[1] blocks
TR_SLOTS = 3         # transpose slots in the trden bank
OC_ENGINES = ("dve",)  # outproj copy engine rotation
QK_ENGINE = "dve"    # qk proj copy engine
V_ENGINE = "dve"     # v proj copy engine
Z2C_ENGINE = "dve"   # transpose psum -> sbuf copy engine
V_IN_TRDEN = False   # V-proj psum in the trden bank vs shared ps_u
INTER_LEAD = 2       # chunk-2 blocks before chunk-3 interleave starts
INTER_N = 6          # chunk-3 blocks interleaved into chunk 2
FMAX = 6             # adaptive pop: keep len(filler) near this
WARMUP = 26          # garbage matmuls before the first DMA lands (pstate ramp)
TR_DELAY = 1         # blocks between a sub's diag PV and its transpose pop
OUT_DELAY = 1        # blocks between transpose and outproj pops


def _build(stage=5):
    nc = bacc.Bacc("TRN2", target_bir_lowering=False, debug=False)

    xT_d = nc.dram_tensor("xT", [E, SEQ], BF16, kind="ExternalInput").ap()
    wq_d = nc.dram_tensor("wq", [128, NKT, H2], BF16, kind="ExternalInput").ap()
    wk_d = nc.dram_tensor("wk", [128, NKT, H2], BF16, kind="ExternalInput").ap()
    wv_d = nc.dram_tensor("wv", [128, NKT, H2], BF16, kind="ExternalInput").ap()
    wo_d = nc.dram_tensor("wo", [H2, E], BF16, kind="ExternalInput").ap()
    masks_d = nc.dram_tensor("masksb", [128, 128], BF16, kind="ExternalInput").ap()
    ident_d = nc.dram_tensor("ident", [128, 128], F32, kind="ExternalInput").ap()
    vcol_d = nc.dram_tensor("vcol", [128, HPC, NCT, 2], BF16,
                            kind="ExternalInput").ap()
    out_d = nc.dram_tensor("out", [SEQ, E], BF16, kind="ExternalOutput").ap()

    def _body(tc):
        with (
            tc.tile_pool(name="persist", bufs=1) as persist,
            tc.tile_pool(name="work", bufs=4) as work,
            tc.tile_pool(name="ps_s", bufs=2, space="PSUM") as ps_s,
            tc.tile_pool(name="ps_zt", bufs=1, space="PSUM") as ps_zt,
            tc.tile_pool(name="ps_p", bufs=1, space="PSUM") as ps_p,
            tc.tile_pool(name="ps_o", bufs=1, space="PSUM") as ps_o,
        ):
            # ---- resident tensors -------------------------------------
            xT = persist.tile([128, NKT, SEQ], BF16)
            wq = persist.tile([128, NKT, H2], BF16)
            wk = persist.tile([128, NKT, H2], BF16)
            wv = persist.tile([128, NKT, H2], BF16)
            wo = persist.tile([128, E], BF16)
            mask_sb = persist.tile([128, 128], BF16)
            ident = persist.tile([128, 128], F32)
            QT2 = persist.tile([128, SEQ], BF16)
            KT2 = persist.tile([128, SEQ], BF16)
            V1 = persist.tile([128, HPC, NCT, H + 2], BF16)

            # persistent PSUM accumulators (allocated once, managed by
            # subtile deps): zT [c, slot, h] and the transpose/denominator
            # bank.  slot = parity*8 + sub*2 + hh.
            # zT accumulator, 2 banks: slot idx = sub*2 + hh, bank idx//4,
            # 65 columns per slot (64 z + the ones-column denominator).
            # start=True zeroes a WHOLE psum bank, so within a PV group only
            # the first matmul touching each bank may use start=True.
            zt = ps_zt.tile([128, 2, CS], F32, name="zt")

            def ztsl(s, hh):
                idx = s * 2 + hh
                return zt[:, idx // 4, (idx % 4) * 65:(idx % 4) * 65 + 65]

            # warmup: garbage matmuls on a never-initialized tile keep the
            # PE busy during the initial DMA wait so the p-state ramp (the
            # cost model halves PE speed for the first 3us of a busy run)
            # is paid on discarded work
            if WARMUP:
                wu = persist.tile([128, 128], BF16)
                nc.gpsimd.memset(wu[:], 0.0)
                wu_ps = ps_p.tile([128, CS], F32, tag="p", name="wu_ps")
                for w in range(WARMUP):
                    nc.tensor.matmul(wu_ps[:, 0:128], wu[:], wu[:],
                                     start=True, stop=True,
                                     skip_group_check=True)

            # ---- input DMAs (issue order tracks first use) ------------
            xTv = xT_d.rearrange("(k p) c -> p k c", p=128)
            nc.sync.dma_start(wq[:, 0:2, :], wq_d[:, 0:2, :])
            nc.sync.dma_start(xT[:, 0:2, 0:CS], xTv[:, 0:2, 0:CS])
            nc.sync.dma_start(wq[:, 2:4, :], wq_d[:, 2:4, :])
            nc.sync.dma_start(xT[:, 2:4, 0:CS], xTv[:, 2:4, 0:CS])
            nc.sync.dma_start(wk[:, 0:4, :], wk_d[:, 0:4, :])
            nc.sync.dma_start(wq[:, NKT // 2:, :], wq_d[:, NKT // 2:, :])
            nc.sync.dma_start(xT[:, NKT // 2:, 0:CS], xTv[:, NKT // 2:, 0:CS])
            nc.sync.dma_start(wk[:, 4:, :], wk_d[:, 4:, :])
            nc.sync.dma_start(wv[:], wv_d[:])
            nc.sync.dma_start(mask_sb[:], masks_d[:])
            nc.sync.dma_start(xT[:, :, CS:2 * CS], xTv[:, :, CS:2 * CS])
            nc.sync.dma_start(V1[:, :, :, H:H + 2], vcol_d[:])
            nc.sync.dma_start(ident[:], ident_d[:])
            nc.sync.dma_start(wo[:], wo_d[:])
            for cc in range(2, NCS):
                nc.sync.dma_start(xT[:, :, cc * CS:(cc + 1) * CS],
                                  xTv[:, :, cc * CS:(cc + 1) * CS])

            def copy_eng(which):
                return {"act": nc.scalar, "dve": nc.vector,
                        "pool": nc.gpsimd}[which]

            def do_copy(which, dst, src):
                if which == "act":
                    nc.scalar.activation(dst, src,
                                         mybir.ActivationFunctionType.Copy)
                else:
                    copy_eng(which).tensor_copy(dst, src)

            # ---- projections ------------------------------------------
            qk_live = {}

            def emit_qk_proj(cc, w_sb, dstT, half, ap_slot=None, defer=False):
                c0, c1 = cc * CS, (cc + 1) * CS
                if half == 0:
                    if ap_slot is None:
                        p_ps = ps_p.tile([128, CS], F32, tag="p", name="p_ps")
                    else:
                        p_ps = ap_slot
                    qk_live[(cc, id(dstT))] = p_ps
                else:
                    p_ps = qk_live.pop((cc, id(dstT)))
                for k in range(4 * half, 4 * half + 4):
                    nc.tensor.matmul(
                        p_ps[:], w_sb[:, k, :], xT[:, k, c0:c1],
                        start=(k == 0), stop=(k == NKT - 1),
                    )
                if half == 1:
                    # the copy pops as its own (delayed) unit so it never
                    # waits at its engine's queue head for the matmuls
                    cp = lambda: do_copy(QK_ENGINE, dstT[:, c0:c1], p_ps[:])
                    if defer:
                        filler.insert(0, (("proj", cc), cur_i[0] + 1, cp))
                    else:
                        cp()

            def emit_v_tile(cc, i, ap_slot=None, defer=False):
                ct = 4 * cc + i
                if ap_slot is not None:
                    v_ps = ap_slot
                elif V_IN_TRDEN:
                    v_ps = vregs[i % 2]
                else:
                    v_ps = ps_p.tile([128, CS], F32, tag="p", name="v_ps")
                for k in range(NKT):
                    nc.tensor.matmul(
                        v_ps[:, 0:H2], xT[:, k, ct * 128:(ct + 1) * 128],
                        wv[:, k, :],
                        start=(k == 0), stop=(k == NKT - 1),
                        skip_group_check=True,
                    )
                cp = lambda: do_copy(V_ENGINE, V1[:, :, ct, 0:H],
                                     v_ps[:, 0:H2])
                if defer:
                    filler.insert(0, (("proj", cc), cur_i[0] + 1, cp))
                else:
                    cp()

            def proj_units(cc, head=False):
                # chunk-0 projections borrow the idle score-psum halves so
                # the initial burst double-buffers; later chunks trickle
                # through the shared 1-bank pool
                units = []
                if head:
                    sa = ps_s.tile([128, 2, CS], F32, tag="s", name="s2")
                    sb = ps_s.tile([128, 2, CS], F32, tag="s", name="s2")
                    vb = ps_o.tile([128, CS], F32, tag="o", name="o_ps")
                    units.append(lambda: emit_qk_proj(cc, wq, QT2, 0,
                                                      sa[:, 0, :]))
                    units.append(lambda: emit_qk_proj(cc, wq, QT2, 1))
                    units.append(lambda: emit_qk_proj(cc, wk, KT2, 0,
                                                      sb[:, 0, :]))
                    units.append(lambda: emit_qk_proj(cc, wk, KT2, 1))
                    units.append(lambda: emit_v_tile(cc, 0, sa[:, 1, :]))
                    units.append(lambda: emit_v_tile(cc, 1, sb[:, 1, :]))
                    units.append(lambda: emit_v_tile(cc, 2, vb))
                    units.append(lambda: emit_v_tile(cc, 3))
                else:
                    units.append(lambda: emit_qk_proj(cc, wq, QT2, 0))
                    units.append(lambda: emit_qk_proj(cc, wq, QT2, 1,
                                                      defer=True))
                    units.append(lambda: emit_qk_proj(cc, wk, KT2, 0))
                    units.append(lambda: emit_qk_proj(cc, wk, KT2, 1,
                                                      defer=True))
                    for i in range(4):
                        units.append(lambda i=i: emit_v_tile(cc, i,
                                                             defer=True))
                return [(("proj", cc), 0, u) for u in units]

            # ---- global block pipeline --------------------------------
            # Score emission order: chunk 3's early C-tiles are interleaved
            # into chunk 2 (chunk 3 is exp-heavy; its scores can run early,
            # buffered in SBUF).  PV order stays natural per chunk so the
            # single-set zT accumulator never sees two chunks at once.
            pv_list = []
            for cs in range(NCS):
                for ct in range(4 * cs + 4):
                    pv_list.append((cs, ct))
            NB = len(pv_list)
            score_list = list(pv_list)
            smap = {b: k for k, b in enumerate(score_list)}

            exp_tiles = {}
            filler = []     # entries: (tag, min_block, fn)
            oc_idx = [0]
            tr_count = [0]
            drain = [False]
            cur_i = [0]

            def do_scores(blk):
                cs, ct = blk
                d = ct - 4 * cs
                off = 128 * d if d > 0 else 0
                n = CS - off
                s2 = ps_s.tile([128, 2, CS], F32, tag="s", name="s2")
                for hh in range(HPC):
                    h0 = hh * H
                    nc.tensor.matmul(
                        s2[:, hh, 0:n],
                        QT2[h0:h0 + H, ct * 128:(ct + 1) * 128],
                        KT2[h0:h0 + H, cs * CS + off:(cs + 1) * CS],
                        start=True, stop=True,
                    )
                e2 = work.tile([128, 2, CS], BF16, tag="exp",
                               bufs=ESB_BUFS, name="e2")
                nc.scalar.activation(
                    e2[:, :, 0:n], s2[:, :, 0:n],
                    mybir.ActivationFunctionType.Exp, scale=SCALE,
                )
                if d >= 0:
                    for hh in range(HPC):
                        nc.gpsimd.tensor_tensor(
                            e2[:, hh, 0:128], e2[:, hh, 0:128], mask_sb[:],
                            op=mybir.AluOpType.mult,
                        )
                exp_tiles[blk] = (e2, off, n)

            def emit_norm(cs, s):
                # recip for this sub-pair group is emitted by do_pv (it only
                # depends on the denominator columns); here: normalize both
                # heads into one [c, h2] sbuf tile, then queue transpose and
                # outproj units
                rsb = norm_recips[cs]
                zn = work.tile([128, H2], F32, tag="zn", bufs=ZN_BUFS,
                               name="zn")
                # one head normalizes on DVE, the other on ACT (activation
                # Copy with a per-partition scale AP) so the two run in
                # parallel -- this chain gates the kernel tail
                nc.vector.tensor_scalar(
                    zn[:, 0:H], ztsl(s, 0)[:, 0:H],
                    rsb[:, s * 2:s * 2 + 1], None,
                    op0=mybir.AluOpType.mult,
                )
                nc.vector.tensor_scalar(
                    zn[:, H:H2], ztsl(s, 1)[:, 0:H],
                    rsb[:, s * 2 + 1:s * 2 + 2], None,
                    op0=mybir.AluOpType.mult,
                )
                filler.insert(0, (("tr", cs), cur_i[0] + TR_DELAY,
                                  make_tr(cs, s, zn)))

            def make_tr(cs, s, zn):
                def run():
                    tr_ps = ps_o.tile([128, CS], F32, tag="o", name="o_ps")
                    nc.tensor.transpose(tr_ps[:, 0:128], zn[:], ident[:])
                    z2 = work.tile([128, 128], BF16, tag="z2", bufs=Z2_BUFS,
                                   name="z2")
                    do_copy("act" if cs == NCS - 1 else Z2C_ENGINE, z2[:],
                            tr_ps[:, 0:128])
                    o_sb = work.tile([128, E], BF16, tag="o", bufs=OSB_BUFS,
                                     name="o_sb")
                    filler.append((("out", cs), cur_i[0] + OUT_DELAY,
                                   make_out(cs, s, z2, 0, o_sb)))
                    filler.append((("out", cs), cur_i[0] + OUT_DELAY + 1,
                                   make_out(cs, s, z2, 1, o_sb)))
                return run

            def make_out(cs, s, z2, et, o_sb):
                def run():
                    last = cs == NCS - 1 and s == 3
                    if drain[0]:
                        # scores are finished: borrow the idle score banks so
                        # the final outproj chain pipelines wider
                        sl = ps_s.tile([128, 2, CS], F32, tag="s", name="s2")
                        o_ps = sl[:, et, :]
                        engs = ("act", "dve")
                    else:
                        o_ps = ps_o.tile([128, CS], F32, tag="o", name="o_ps")
                        engs = OC_ENGINES
                    nc.tensor.matmul(
                        o_ps[:], z2[:], wo[:, et * CS:(et + 1) * CS],
                        start=True, stop=True,
                    )
                    eng = engs[oc_idx[0] % len(engs)]
                    oc_idx[0] += 1
                    do_copy(eng, o_sb[:, et * CS:(et + 1) * CS], o_ps[:])
                    if last:
                        nc.sync.dma_start(
                            out_d[cs * CS + s * 128:cs * CS + (s + 1) * 128,
                                  et * CS:(et + 1) * CS],
                            o_sb[:, et * CS:(et + 1) * CS])
                    elif et == 1:
                        nc.sync.dma_start(
                            out_d[cs * CS + s * 128:cs * CS + (s + 1) * 128, :],
                            o_sb[:])
                return run

            norm_recips = {}

            def do_pv(blk):
                cs, ct = blk
                d = ct - 4 * cs
                off = 128 * d if d > 0 else 0
                diag = ct - 4 * cs  # sub index whose accumulation ends here
                # masked sub (s == d) last: its lhsT waits the DVE mask op
                subs = list(range(max(d, 0), 4))
                if d >= 0 and len(subs) > 1:
                    subs = subs[1:] + subs[:1]
                e2, off_, n = exp_tiles.pop(blk)
                bank_started = [False, False]
                for hh in range(HPC):
                    for s in subs:
                        lo = s * 128 - off
                        bank = (s * 2 + hh) // 4
                        st = False
                        if ct == 0 and not bank_started[bank]:
                            st = True
                            bank_started[bank] = True
                        nc.tensor.matmul(
                            ztsl(s, hh), e2[:, hh, lo:lo + 128],
                            V1[:, hh, ct, 0:H + 1],
                            start=st, stop=(ct == 4 * cs + s),
                            skip_group_check=True,
                        )
                if 0 <= diag < 4:
                    s = diag
                    # reciprocal granularity: subs {0,1} together, then 2, 3
                    groups = {1: (0, 1), 2: (2,), 3: (3,)}
                    if cs == 0 or cs == NCS - 1:
                        groups = {0: (0,), 1: (1,), 2: (2,), 3: (3,)}
                    if s in groups:
                        g = groups[s]
                        lo_s, hi_s = g[0], g[-1]
                        rsb = norm_recips.get(cs)
                        if rsb is None:
                            rsb = work.tile([128, 8], F32, tag="recip",
                                            bufs=2, name="rsb")
                            norm_recips[cs] = rsb
                        for ss in g:
                            for hh in range(HPC):
                                nc.vector.reciprocal(
                                    rsb[:, ss * 2 + hh:ss * 2 + hh + 1],
                                    ztsl(ss, hh)[:, 64:65])
                        for ss in g:
                            emit_norm(cs, ss)

            def pop(nmax=1):
                npop = 0
                j = 0
                while j < len(filler) and npop < nmax:
                    tag, min_i, fn = filler[j]
                    if drain[0] or cur_i[0] >= min_i:
                        filler.pop(j)
                        fn()
                        npop += 1
                    else:
                        j += 1

            def force_proj(cs):
                j = 0
                while j < len(filler):
                    if filler[j][0] == ("proj", cs):
                        filler.pop(j)[2]()
                    else:
                        j += 1

            for i in range(NB):
                cur_i[0] = i
                cs, ct = pv_list[i]
                if ct == 0:
                    force_proj(cs)
                    if cs == 0:
                        for _, _, u in proj_units(0, head=True):
                            u()
                        filler.extend(proj_units(1))
                    elif cs == 1:
                        filler.extend(proj_units(2))
                    elif cs == 2:
                        filler.extend(proj_units(3))
                do_scores(pv_list[i])
                npop = POPN[0]
                if len(filler) > FMAX:
                    npop += (len(filler) - FMAX + 1) // 2
                pop(npop)
                if i >= DEPTH:
                    do_pv(pv_list[i - DEPTH])
            drain[0] = True
            for i in range(NB - DEPTH, NB):
                do_pv(pv_list[i])
                pop(2)
            while filler:
                pop(1)

    with tile.TileContext(nc) as tc:
        _body(tc)
    nc.finalize()
    return nc


def _prep_inputs(x, W_Q, W_K, W_V, W_O):
    x = np.asarray(x, dtype=np.float32)
    W_Q = np.asarray(W_Q, dtype=np.float32)
    W_K = np.asarray(W_K, dtype=np.float32)
    W_V = np.asarray(W_V, dtype=np.float32)
    W_O = np.asarray(W_O, dtype=np.float32)

    xT = np.ascontiguousarray(x[0].T).astype(NPBF16)       # [E, SEQ]

    def swz(w):
        # [E, H2] -> [128(p), NKT(k), H2]
        return np.ascontiguousarray(
            w.reshape(NKT, 128, H2).transpose(1, 0, 2)).astype(NPBF16)

    in_maps = []
    for c in range(NCORES):
        a0, a1 = HPC * c, HPC * c + 1
        wq = swz(np.concatenate([W_Q[a0].T, W_Q[a1].T], axis=1))
        wk = swz(np.concatenate([W_K[a0].T, W_K[a1].T], axis=1))
        wv = swz(np.concatenate([W_V[a0].T, W_V[a1].T], axis=1))
        wo = np.ascontiguousarray(
            np.concatenate([W_O[a0].T, W_O[a1].T], axis=0)).astype(NPBF16)
        in_maps.append({"xT": xT, "wq": wq, "wk": wk, "wv": wv, "wo": wo,
                        "masksb": _MASKS, "ident": _IDENT, "vcol": _VCOL})
    return in_maps


_MASKS = (np.arange(128)[:, None] <= np.arange(128)[None, :]).astype(NPBF16)
_IDENT = np.eye(128, dtype=np.float32)
_VCOL = np.ones((128, HPC, NCT, 2), dtype=NPBF16)


def _run(in_maps, trace=False):
    global _built
    if _built is None:
        _built = _build()
    res = bass_utils.run_bass_kernel_spmd(
        _built, in_maps, core_ids=list(range(NCORES)), trace=trace,
    )
    return res


def kernel(x, W_Q, W_K, W_V, W_O):
    in_maps = _prep_inputs(x, W_Q, W_K, W_V, W_O)
    res = _run(in_maps, trace=False)
    acc = np.zeros((SEQ, E), dtype=np.float64)
    for c in range(NCORES):
        acc += np.asarray(res.results[c]["out"], dtype=np.float64)
    return acc.astype(np.float32)[None, :, :]


def kernel_traced(x, W_Q, W_K, W_V, W_O):
    """Like kernel() but also returns a per-core exec-time estimate in ns."""
    in_maps = _prep_inputs(x, W_Q, W_K, W_V, W_O)
    exec_ns = None
    try:
        res = _run(in_maps, trace=True)
        exec_ns = res.exec_time_ns
    except Exception:
        res = _run(in_maps, trace=False)
    if exec_ns is None:
        from concourse.timeline_sim import TimelineSim
        exec_ns = int(TimelineSim(_built, trace=False).simulate())
    acc = np.zeros((SEQ, E), dtype=np.float64)
    for c in range(NCORES):
        acc += np.asarray(res.results[c]["out"], dtype=np.float64)
    return acc.astype(np.float32)[None, :, :], exec_ns
